# revision 18
# baseline (speedup 1.0000x reference)
"""Trainium2 Bass kernel for nn_MemPIDModel (dense_cnn) — sequence-parallel.

Strategy (8 NeuronCores):
  - core c handles sample b = c//4, token quarter q = c%4 (256 tokens each)
  - trunk is sequence-parallel: per layer, each core computes its 256-token
    slice; the causal dilated conv's halo is exchanged via a 4-rank
    AllGather of the normed conv input (bf16), sized to the layer's actual
    receptive field h = min(14*dil, 256) tokens
  - halo readback: indirect DMA with per-core row-offset tables (an
    always-zero block in each AG buffer provides causal zero padding, so
    one SPMD program works for all cores); full-row gathers land strided
    across the 4 d-tiles in one instruction
  - conv runs fully on PE as 15 diag-matmul accumulations per d-tile at
    256-col streams (matmul issue cadence bound -> widest possible)
  - mix boundaries: local cumsum scan + tiny AllGather of slice totals for
    the cross-core prefix (mask-weighted sum keeps the program uniform)
  - head: AllGather of the normed trunk output, then each core computes its
    vocab shard v = c%4 over all 1024 tokens ([1024,512]@[512,8000])
"""

import os
import sys
import numpy as np

sys.path.insert(0, "/opt/trn_rl_repo")

import ml_dtypes

B = 2
T = 1024
D = 512
HID = 1024
KK = 15
VOCAB = 32000
RANK = 64
NL = 6
MIX_W = 0.1
UP_DIL = [1, 2, 4, 8, 16, 32]
DN_DIL = UP_DIL[::-1]
EPS = 1e-6
NCORES = 8
VSHARDS = 4
VS = VOCAB // VSHARDS  # 8000
DT = D // 128  # 4 D-tiles
HT = HID // 128  # 8 H-tiles
TO = 256  # own tokens per core
POFF = 512  # own region offset inside xnb (per d-tile)
CONVW = POFF + TO  # 768 padded conv input width per d-tile
NV = (VS + 511) // 512  # 16 head column chunks (last is 320 wide)
HS = sorted({min(14 * d, 256) for d in UP_DIL})  # AG payload widths

BF16 = ml_dtypes.bfloat16

# debug knobs (affect program shape; kernel cache key includes them)
N_LAYERS = int(os.environ.get("KB_LAYERS", str(NL)))
N_STACKS = int(os.environ.get("KB_STACKS", "3"))
SKIP_HEAD = bool(int(os.environ.get("KB_SKIP_HEAD", "0")))
DEBUG_TRUNK_OUT = bool(int(os.environ.get("KB_TRUNK_OUT", "0")))

_prog_cache = {}


def _f32(x):
    return np.ascontiguousarray(np.asarray(x), dtype=np.float32)


def _cols(v):
    """[D] vector -> [128, DT] A-layout per-partition columns."""
    return np.ascontiguousarray(_f32(v).reshape(DT, 128).T)


def _stack_dils(stack_idx):
    return UP_DIL if stack_idx in (0, 2) else DN_DIL


def build_program():
    key = (N_LAYERS, N_STACKS, SKIP_HEAD, DEBUG_TRUNK_OUT)
    if key in _prog_cache:
        return _prog_cache[key]

    import concourse.bass as bass
    import concourse.mybir as mybir
    import concourse.tile as tile
    from concourse import bacc
    from concourse.masks import make_identity

    dt = mybir.dt
    Alu = mybir.AluOpType
    Act = mybir.ActivationFunctionType

    nc = bacc.Bacc(None, target_bir_lowering=False, debug=False)

    RG = [[0, 1, 2, 3], [4, 5, 6, 7]]

    # ---------------- DRAM I/O ----------------
    d_idx = nc.dram_tensor("idx_rs", [128, 3], dt.int32, kind="ExternalInput")
    d_emb = nc.dram_tensor("emb_tbl", [VOCAB, D], dt.float32, kind="ExternalInput")
    d_pos = nc.dram_tensor("pos_rs", [128, 3 * D], dt.float32, kind="ExternalInput")
    d_cst = nc.dram_tensor("cst", [128, 64], dt.float32, kind="ExternalInput")
    d_rc = nc.dram_tensor("rc_bc", [128, TO], dt.float32, kind="ExternalInput")
    d_mgw = nc.dram_tensor("mgwT", [D, D], dt.float32, kind="ExternalInput")
    d_offs = nc.dram_tensor("halo_offs", [128, 2], dt.int32, kind="ExternalInput")

    d_cwb = {}
    d_cwt = {}
    d_w13 = {}
    d_w2 = {}
    d_pid = {}
    d_cdiag = {}
    d_gdiag = {}
    for s in ("up", "dn"):
        d_cwb[s] = nc.dram_tensor(
            f"{s}_cwb", [NL, 128, DT], dt.float32, kind="ExternalInput"
        )
        d_cwt[s] = nc.dram_tensor(
            f"{s}_cwt", [NL, 128, KK], dt.float32, kind="ExternalInput"
        )
        d_w13[s] = nc.dram_tensor(
            f"{s}_w13p", [NL, 128, DT * 2 * HID], dt.bfloat16, kind="ExternalInput"
        )
        d_w2[s] = nc.dram_tensor(
            f"{s}_w2p", [NL, 128, HT * D], dt.bfloat16, kind="ExternalInput"
        )
        d_pid[s] = nc.dram_tensor(
            f"{s}_pid", [128, (NL - 1) * 12], dt.float32, kind="ExternalInput"
        )
        d_cdiag[s] = nc.dram_tensor(
            f"{s}_cdiag", [NL, 128, DT * KK * 128], dt.bfloat16, kind="ExternalInput"
        )
        d_gdiag[s] = nc.dram_tensor(
            f"{s}_gdiag", [NL, 128, DT * 128], dt.bfloat16, kind="ExternalInput"
        )
    d_dwT = nc.dram_tensor("sg_dwT", [3, D, RANK], dt.bfloat16, kind="ExternalInput")
    d_uwT = nc.dram_tensor("sg_uwT", [3, RANK, D], dt.bfloat16, kind="ExternalInput")
    d_sgc = nc.dram_tensor("sg_cols", [128, 16], dt.float32, kind="ExternalInput")
    d_embT = nc.dram_tensor("embT_sh", [NV, 128, DT * 512], dt.bfloat16, kind="ExternalInput")

    d_out = nc.dram_tensor("logits_sh", [T, VS], dt.float32, kind="ExternalOutput")
    if DEBUG_TRUNK_OUT:
        d_trunk = nc.dram_tensor("trunk_out", [128, DT * TO], dt.float32, kind="ExternalOutput")

    f32 = dt.float32
    bf = dt.bfloat16

    with tile.TileContext(nc) as tc:
        import contextlib

        ctx = contextlib.ExitStack()
        with ctx:
            const = ctx.enter_context(tc.tile_pool(name="const", bufs=1))
            master = ctx.enter_context(tc.tile_pool(name="master", bufs=1))
            lay = ctx.enter_context(tc.tile_pool(name="lay", bufs=1))
            wgt = ctx.enter_context(tc.tile_pool(name="wgt", bufs=2))
            psum = ctx.enter_context(tc.tile_pool(name="psum", bufs=1, space="PSUM"))
            dram = ctx.enter_context(tc.tile_pool(name="dram", bufs=1, space="DRAM"))

            # ---------------- constants ----------------
            epsc = const.tile([128, 1], f32, tag="epsc")
            nc.vector.memset(epsc[:], EPS)
            ones_bf = const.tile([128, 1], bf, tag="ones")
            nc.vector.memset(ones_bf[:], 1.0)
            ones_row = const.tile([1, 128], bf, tag="onesr")
            nc.vector.memset(ones_row[:], 1.0)
            ident = const.tile([128, 128], f32, tag="ident")
            make_identity(nc, ident[:])
            cst = const.tile([128, 64], f32, tag="cst")
            nc.sync.dma_start(cst[:], d_cst[:])
            rc_bc = const.tile([128, TO], f32, tag="rc")
            nc.sync.dma_start(rc_bc[:], d_rc[:])
            sgc = const.tile([128, 16], f32, tag="sgc")
            nc.sync.dma_start(sgc[:], d_sgc[:])
            offs = const.tile([128, 2], dt.int32, tag="offs")
            nc.sync.dma_start(offs[:], d_offs[:])
            pidc = {}
            for s in ("up", "dn"):
                pidc[s] = const.tile(
                    [128, (NL - 1) * 12], f32, tag=f"pid_{s}", name=f"pid_{s}"
                )
                nc.sync.dma_start(pidc[s][:], d_pid[s][:])

            # AllGather buffers per payload width h: out rows [0:512] written
            # by AG, rows [512:640] stay zero (causal pad source)
            zero_sb = const.tile([128, DT * TO], bf, tag="zsb")
            nc.gpsimd.memset(zero_sb[:], 0.0)
            agiT = {}
            agoT = {}
            for h in HS:
                agiT[h] = dram.tile([128, DT * h], bf, tag=f"agi{h}", name=f"agi{h}")
                agoT[h] = dram.tile([5 * 128, DT * h], bf, tag=f"ago{h}", name=f"ago{h}")
                nc.sync.dma_start(agoT[h][4 * 128 : 5 * 128, :], zero_sb[:, 0 : DT * h])
            bs_in = dram.tile([128, DT], f32, tag="bs_in", bufs=1)
            bs_out = dram.tile([4 * 128, DT], f32, tag="bs_out", bufs=1)

            # persistent activations (A-layout, free index = dt*TO + t)
            xA = master.tile([128, DT * TO], f32, tag="xA")

            def keep_tile():  # initial, then gated2 (sequential lifetimes)
                return master.tile([128, DT * TO], f32, tag="keep", name="keep")

            def f32a_tile():  # integ during stacks / mixed during boundaries
                return lay.tile([128, DT * TO], f32, tag="f32a", name="f32a")

            def t_zb():
                return lay.tile([128, DT * TO], bf, tag="zb", name="zb", bufs=2)

            def t_ub():
                return lay.tile([128, DT * TO], bf, tag="ub", name="ub", bufs=2)

            def t_sq():
                return lay.tile([128, DT * TO], bf, tag="sq", name="sq", bufs=2)

            def t_hb():
                return lay.tile([128, DT * TO], bf, tag="hb", name="hb", bufs=2)

            def t_sbc():
                return lay.tile([128, TO], bf, tag="sbc", name="sbc", bufs=2)

            def t_srow():
                return lay.tile([1, TO], bf, tag="srow", name="srow", bufs=2)

            def t_pch():
                return lay.tile([128, HT * TO], bf, tag="pch", name="pch", bufs=2)

            def t_gst():
                return lay.tile([128, 2 * TO], bf, tag="gst", name="gst", bufs=2)

            def t_xnb():
                return lay.tile([128, DT * CONVW], bf, tag="xnb", name="xnb", bufs=2)

            # PSUM: pc [128,1024]=2 banks, pg/pu/py [*,<=512] bufs=2
            def pc_tile():
                return psum.tile([128, DT * TO], f32, tag="pc", bufs=1, name="pc")

            def pg_tile():
                return psum.tile([128, 2 * TO], f32, tag="pg", bufs=2, name="pg")

            def pu_tile():
                return psum.tile([128, 2 * TO], f32, tag="pu", bufs=2, name="pu")

            def py_tile():
                return psum.tile([128, TO], f32, tag="py", bufs=2, name="py")

            # ---------------- P0: gather + embnorm + shift + mem ----------------
            with tc.tile_pool(name="p0", bufs=1) as p0:
                idx_sb = p0.tile([128, 3], dt.int32, tag="idx")
                nc.sync.dma_start(idx_sb[:], d_idx[:])
                gth = p0.tile([128, 3 * D], f32, tag="gth")
                for c in range(3):
                    nc.gpsimd.indirect_dma_start(
                        out=gth[:, c * D : (c + 1) * D],
                        out_offset=None,
                        in_=d_emb[:],
                        in_offset=bass.IndirectOffsetOnAxis(ap=idx_sb[:, c : c + 1], axis=0),
                    )
                pos_sb = p0.tile([128, 3 * D], f32, tag="pos")
                nc.sync.dma_start(pos_sb[:], d_pos[:])
                nc.vector.tensor_tensor(
                    out=gth[:], in0=gth[:], in1=pos_sb[:], op=Alu.add
                )
                ss = p0.tile([128, 3], f32, tag="ss")
                sqt = p0.tile([128, D], f32, tag="sqt")
                for c in range(3):
                    nc.scalar.activation(
                        sqt[:],
                        gth[:, c * D : (c + 1) * D],
                        Act.Square,
                        accum_out=ss[:, c : c + 1],
                    )
                nc.scalar.activation(ss[:], ss[:], Act.Ln, bias=epsc[:], scale=1.0 / D)
                nc.scalar.activation(ss[:], ss[:], Act.Exp, scale=-0.5)
                for c in range(3):
                    nc.vector.tensor_scalar(
                        gth[:, c * D : (c + 1) * D],
                        gth[:, c * D : (c + 1) * D],
                        ss[:, c : c + 1],
                        None,
                        Alu.mult,
                    )
                # transpose token-major -> A-layout x_n [128, DT*384]
                x_n = p0.tile([128, DT * 384], f32, tag="xn")
                for c in range(3):
                    pst = pg_tile()
                    for dtt in range(2):
                        nc.tensor.transpose(
                            out=pst[:, dtt * 128 : (dtt + 1) * 128],
                            in_=gth[:, c * D + dtt * 128 : c * D + (dtt + 1) * 128],
                            identity=ident[:],
                        )
                    pst2 = pu_tile()
                    for dtt in range(2):
                        nc.tensor.transpose(
                            out=pst2[:, dtt * 128 : (dtt + 1) * 128],
                            in_=gth[:, c * D + (2 + dtt) * 128 : c * D + (3 + dtt) * 128],
                            identity=ident[:],
                        )
                    for dtt in range(2):
                        nc.vector.tensor_copy(
                            x_n[:, dtt * 384 + c * 128 : dtt * 384 + (c + 1) * 128],
                            pst[:, dtt * 128 : (dtt + 1) * 128],
                        )
                        nc.vector.tensor_copy(
                            x_n[:, (2 + dtt) * 384 + c * 128 : (2 + dtt) * 384 + (c + 1) * 128],
                            pst2[:, dtt * 128 : (dtt + 1) * 128],
                        )
                # mem gate: mem = sigmoid(memp @ mgw.T + mgb)
                ps_mem = py_tile()
                mgw_sb = p0.tile([128, DT * D], f32, tag="mgw", name="mgw_sb")
                nc.scalar.dma_start(
                    mgw_sb[:].rearrange("p (k d) -> p k d", k=DT),
                    d_mgw[:].rearrange("(k p) d -> p k d", p=128),
                )
                for kt in range(DT):
                    for m in range(DT):
                        nc.tensor.matmul(
                            ps_mem[:, m : m + 1],
                            lhsT=mgw_sb[:, kt * D + m * 128 : kt * D + (m + 1) * 128],
                            rhs=cst[:, 20 + kt : 21 + kt],
                            start=(kt == 0),
                            stop=(kt == DT - 1),
                        )
                tmem = p0.tile([128, 4], f32, tag="tmem")
                for m in range(DT):
                    nc.scalar.activation(
                        tmem[:, m : m + 1],
                        ps_mem[:, m : m + 1],
                        Act.Tanh,
                        scale=0.5,
                        bias=cst[:, 16 + m : 17 + m],
                    )
                nc.vector.tensor_scalar(tmem[:], tmem[:], 0.5, 0.5, Alu.mult, Alu.add)
                # token shift + mem (own tokens live at x_n cols 128..384)
                tsh = p0.tile([128, TO], f32, tag="tsh")
                for dtt in range(DT):
                    ox = dtt * 384 + 128  # own region in x_n
                    oa = dtt * TO  # xA
                    nc.vector.tensor_scalar(
                        tsh[:, 0:1], x_n[:, ox - 1 : ox], cst[:, 12 + dtt : 13 + dtt],
                        None, Alu.mult,
                    )
                    nc.vector.scalar_tensor_tensor(
                        out=xA[:, oa : oa + 1],
                        in0=x_n[:, ox : ox + 1],
                        scalar=cst[:, dtt : dtt + 1],
                        in1=tsh[:, 0:1],
                        op0=Alu.mult,
                        op1=Alu.add,
                    )
                    nc.vector.tensor_scalar(
                        tsh[:, 1:TO],
                        x_n[:, ox + 1 : ox + TO],
                        cst[:, 8 + dtt : 9 + dtt],
                        None,
                        Alu.mult,
                    )
                    nc.vector.scalar_tensor_tensor(
                        out=xA[:, oa + 1 : oa + TO],
                        in0=x_n[:, ox : ox + TO - 1],
                        scalar=cst[:, 4 + dtt : 5 + dtt],
                        in1=tsh[:, 1:TO],
                        op0=Alu.mult,
                        op1=Alu.add,
                    )
                    nc.vector.tensor_scalar(
                        xA[:, oa : oa + TO], xA[:, oa : oa + TO],
                        tmem[:, dtt : dtt + 1], None, Alu.add,
                    )
            initial = keep_tile()
            nc.vector.tensor_scalar(initial[:], xA[:], 1.0, None, Alu.mult)

            # ---------------- conv block stack ----------------
            def load_layer_weights(s, li, q2=None):
                w = {}
                if q2 is None:
                    q2 = nc.scalar
                w["cwb"] = wgt.tile([128, DT], f32, tag="cwb", name="cwb")
                nc.sync.dma_start(w["cwb"][:], d_cwb[s][li])
                w["cwt"] = wgt.tile([128, KK], f32, tag="cwt", name="cwt")
                nc.sync.dma_start(w["cwt"][:], d_cwt[s][li])
                w["w13"] = wgt.tile([128, DT * 2 * HID], bf, tag="w13", name="w13")
                for qq in range(2):
                    qs = qq * 4 * HID
                    nc.sync.dma_start(
                        w["w13"][:, qs : qs + 4 * HID], d_w13[s][li, :, qs : qs + 4 * HID]
                    )
                w["w2"] = wgt.tile([128, HT * D], bf, tag="w2", name="w2")
                q2.dma_start(w["w2"][:], d_w2[s][li])
                w["cdiag"] = wgt.tile([128, DT * KK * 128], bf, tag="cdiag", name="cdiag")
                q2.dma_start(w["cdiag"][:], d_cdiag[s][li])
                w["gdiag"] = wgt.tile([128, DT * 128], bf, tag="gdiag", name="gdiag")
                q2.dma_start(w["gdiag"][:], d_gdiag[s][li])
                return w

            def run_stack(stack_idx):
                s = "up" if stack_idx in (0, 2) else "dn"
                dils = _stack_dils(stack_idx)
                integ = f32a_tile()
                nc.vector.tensor_scalar(integ[:], xA[:], 1.0, None, Alu.mult)

                wts = {}
                for li in range(min(2, N_LAYERS)):
                    wts[li] = load_layer_weights(
                        s, li, q2=(nc.sync if li == 0 else nc.scalar)
                    )

                st = {}

                def S1(li):
                    # rmsnorm scale -> xnb own region (ub/sq made by S3 tail
                    # of the previous layer for li>0)
                    cs = st.setdefault(li, {})
                    if li == 0:
                        ub = cs["ub"] = t_ub()
                        nc.scalar.activation(ub[:], xA[:], Act.Copy)
                        sq = cs["sq"] = t_sq()
                        nc.scalar.activation(sq[:], ub[:], Act.Square)
                    ub = cs["ub"]
                    sq = cs["sq"]
                    pn = py_tile()
                    for kt in range(DT):
                        nc.tensor.matmul(
                            pn[0:1, :],
                            lhsT=ones_bf[:],
                            rhs=sq[:, kt * TO : (kt + 1) * TO],
                            start=(kt == 0),
                            stop=(kt == DT - 1),
                        )
                    srow = t_srow()
                    nc.scalar.activation(
                        srow[:], pn[0:1, :], Act.Abs_reciprocal_sqrt,
                        bias=epsc[0:1, :], scale=1.0 / D,
                    )
                    # broadcast via 1-row PE matmul (keeps gpsimd queue clear)
                    sbc = py_tile()
                    nc.tensor.matmul(
                        sbc[:], lhsT=ones_row[0:1, :], rhs=srow[0:1, :],
                        start=True, stop=True,
                    )
                    xnb = st[("xnb", li)]
                    for dtt in range(DT):
                        nc.vector.tensor_tensor(
                            out=xnb[:, dtt * CONVW + POFF : dtt * CONVW + POFF + TO],
                            in0=ub[:, dtt * TO : (dtt + 1) * TO],
                            in1=sbc[:],
                            op=Alu.mult,
                        )

                def AG(li):
                    # exchange last h own tokens; read back h-token halo
                    d = dils[li]
                    h = min(14 * d, TO)
                    xnb = st[("xnb", li)]
                    agi, ago = agiT[h], agoT[h]
                    xnb3 = xnb[:].rearrange("p (d w) -> p d w", d=DT)
                    nc.sync.dma_start(agi[:], xnb3[:, :, POFF + TO - h : POFF + TO])
                    nc.gpsimd.collective_compute(
                        "AllGather",
                        mybir.AluOpType.bypass,
                        replica_groups=RG,
                        ins=[agi[:]],
                        outs=[ago[0 : 4 * 128, :]],
                    )
                    # prev1: its last h tokens -> staging -> xnb [POFF-h, POFF)
                    stg = lay.tile([128, DT * TO], bf, tag="stg", name="stg", bufs=2)
                    nc.gpsimd.indirect_dma_start(
                        out=stg[:, 0 : DT * h],
                        out_offset=None,
                        in_=ago[:],
                        in_offset=bass.IndirectOffsetOnAxis(ap=offs[:, 0:1], axis=0),
                        element_offset=0,
                    )
                    stg3 = stg[:, 0 : DT * h].rearrange("p (d w) -> p d w", d=DT)
                    nc.vector.tensor_copy(xnb3[:, :, POFF - h : POFF], stg3[:, :, :])
                    if 14 * d > TO:
                        # d=32: prev2 tokens [beta-448,beta-256) via its cols [64,256)
                        stg2 = lay.tile([128, DT * TO], bf, tag="stg", name="stg2", bufs=2)
                        nc.gpsimd.indirect_dma_start(
                            out=stg2[:, 0 : DT * h],
                            out_offset=None,
                            in_=ago[:],
                            in_offset=bass.IndirectOffsetOnAxis(ap=offs[:, 1:2], axis=0),
                            element_offset=0,
                        )
                        stg23 = stg2[:, 0 : DT * h].rearrange("p (d w) -> p d w", d=DT)
                        nc.vector.tensor_copy(
                            xnb3[:, :, 64:TO], stg23[:, :, 64:TO]
                        )

                def S2(li):
                    # conv: dtiles 0-2 on PE (diag matmuls, own/halo col-split
                    # for d>=16 so own-col work runs during the AllGather);
                    # dtile 3 on Pool as an STT tap chain
                    d = dils[li]
                    cs = st[li]
                    w = wts[li]
                    xnb = st[("xnb", li)]
                    cacc = lay.tile([128, TO], bf, tag="cacc", name="cacc", bufs=2)
                    ob3 = 3 * CONVW

                    def tap_in(m):
                        stt = ob3 + POFF - m * d
                        return xnb[:, stt : stt + TO]

                    nc.vector.tensor_scalar(
                        cacc[:], tap_in(KK - 1), w["cwt"][:, KK - 1 : KK], None, Alu.mult
                    )
                    for m in range(KK - 2, -1, -1):
                        nc.vector.scalar_tensor_tensor(
                            out=cacc[:],
                            in0=tap_in(m),
                            scalar=w["cwt"][:, m : m + 1],
                            in1=cacc[:],
                            op0=Alu.mult,
                            op1=Alu.add,
                        )
                    psc = pc_tile()
                    for dtt in range(3):
                        ob = dtt * CONVW
                        oc = dtt * TO
                        if d >= 16:
                            # own-col parts first (no halo dependency)
                            for m in range(KK):
                                lo = m * d
                                if lo >= TO:
                                    continue
                                nc.tensor.matmul(
                                    psc[:, oc + lo : oc + TO],
                                    lhsT=w["cdiag"][:, (dtt * KK + m) * 128 : (dtt * KK + m + 1) * 128],
                                    rhs=xnb[:, ob + POFF : ob + POFF + TO - lo],
                                    start=(m == 0),
                                    stop=(m == KK - 1),
                                )
                            for m in range(1, KK):
                                lo = min(m * d, TO)
                                nc.tensor.matmul(
                                    psc[:, oc : oc + lo],
                                    lhsT=w["cdiag"][:, (dtt * KK + m) * 128 : (dtt * KK + m + 1) * 128],
                                    rhs=xnb[:, ob + POFF - m * d : ob + POFF - m * d + lo],
                                    start=False,
                                    stop=(m == KK - 1),
                                )
                        else:
                            for m in range(KK - 1, -1, -1):
                                stt = ob + POFF - m * d
                                nc.tensor.matmul(
                                    psc[:, oc : oc + TO],
                                    lhsT=w["cdiag"][:, (dtt * KK + m) * 128 : (dtt * KK + m + 1) * 128],
                                    rhs=xnb[:, stt : stt + TO],
                                    start=(m == KK - 1),
                                    stop=(m == 0),
                                )
                    hb = cs["hb"] = t_hb()
                    for dtt in range(3):
                        nc.scalar.activation(
                            hb[:, dtt * TO : (dtt + 1) * TO],
                            psc[:, dtt * TO : (dtt + 1) * TO],
                            Act.Silu,
                            bias=w["cwb"][:, dtt : dtt + 1],
                        )
                    nc.scalar.activation(
                        hb[:, 3 * TO : 4 * TO],
                        cacc[:],
                        Act.Silu,
                        bias=w["cwb"][:, 3:4],
                    )

                def S3(li):
                    cs = st[li]
                    w = wts[li]
                    xnb = st[("xnb", li)]
                    hb = cs["hb"]
                    pch = t_pch()
                    for pr in range(4):
                        pg = pg_tile()
                        for j in range(2):
                            kh = pr * 2 + j
                            for kt in range(DT):
                                nc.tensor.matmul(
                                    pg[:, j * TO : (j + 1) * TO],
                                    lhsT=w["w13"][:, kt * 2 * HID + kh * 128 : kt * 2 * HID + (kh + 1) * 128],
                                    rhs=hb[:, kt * TO : (kt + 1) * TO],
                                    start=(kt == 0),
                                    stop=(kt == DT - 1),
                                )
                        gst = t_gst()
                        nc.scalar.activation(gst[:], pg[:], Act.Silu)
                        pu = pu_tile()
                        for j in range(2):
                            kh = pr * 2 + j
                            for kt in range(DT):
                                nc.tensor.matmul(
                                    pu[:, j * TO : (j + 1) * TO],
                                    lhsT=w["w13"][:, kt * 2 * HID + HID + kh * 128 : kt * 2 * HID + HID + (kh + 1) * 128],
                                    rhs=hb[:, kt * TO : (kt + 1) * TO],
                                    start=(kt == 0),
                                    stop=(kt == DT - 1),
                                )
                        nc.vector.tensor_tensor(
                            out=pch[:, pr * 2 * TO : (pr + 1) * 2 * TO],
                            in0=gst[:],
                            in1=pu[:],
                            op=Alu.mult,
                        )
                    for md in range(DT):
                        py = py_tile()
                        if li > 0:
                            base = xnb[:, md * CONVW + POFF : md * CONVW + POFF + TO]
                        else:
                            base = cs["ub"][:, md * TO : (md + 1) * TO]
                        nc.tensor.matmul(
                            py[:],
                            lhsT=w["gdiag"][:, md * 128 : (md + 1) * 128],
                            rhs=base,
                            start=True,
                            stop=False,
                        )
                        for kh in range(HT):
                            nc.tensor.matmul(
                                py[:],
                                lhsT=w["w2"][:, kh * D + md * 128 : kh * D + (md + 1) * 128],
                                rhs=pch[:, kh * TO : (kh + 1) * TO],
                                start=False,
                                stop=(kh == HT - 1),
                            )
                        xs = xA[:, md * TO : (md + 1) * TO]
                        if md % 2 == 0:
                            nc.scalar.activation(xs, py[:], Act.Copy)
                        else:
                            nc.vector.tensor_copy(xs, py[:])
                    # tail: per d-tile integ update + next layer's PID/silu/sq
                    if li < N_LAYERS - 1:
                        pc = pidc[s]
                        pb = li * 12
                        zb = t_zb()
                        nxt = st.setdefault(li + 1, {})
                        ub2 = nxt["ub"] = t_ub()
                        sq2 = nxt["sq"] = t_sq()
                        for dtt in range(DT):
                            sl = slice(dtt * TO, (dtt + 1) * TO)
                            # zb = (kp+ki')*xA + ki'*integ_old
                            nc.vector.tensor_scalar(
                                zb[:, sl], xA[:, sl],
                                pc[:, pb + dtt : pb + 1 + dtt], None, Alu.mult,
                            )
                            nc.vector.scalar_tensor_tensor(
                                out=zb[:, sl],
                                in0=integ[:, sl],
                                scalar=pc[:, pb + 4 + dtt : pb + 5 + dtt],
                                in1=zb[:, sl],
                                op0=Alu.mult,
                                op1=Alu.add,
                            )
                            nc.gpsimd.tensor_tensor(
                                out=integ[:, sl], in0=integ[:, sl],
                                in1=xA[:, sl], op=Alu.add,
                            )
                            nc.scalar.activation(
                                ub2[:, sl], zb[:, sl], Act.Silu
                            )
                            nc.scalar.activation(
                                sq2[:, sl], ub2[:, sl], Act.Square
                            )

                for li in range(N_LAYERS):
                    st[("xnb", li)] = t_xnb()
                    S1(li)
                    AG(li)
                    S2(li)
                    S3(li)
                    if li + 2 < N_LAYERS:
                        wts[li + 2] = load_layer_weights(s, li + 2)
                    st.pop(li, None)
                    st.pop(("xnb", li), None)
                    wts.pop(li, None)

            # ---------------- mix + sgate boundary ----------------
            def boundary(k, old_tile):
                mixed = f32a_tile()  # integ dead
                cs = lay.tile([128, DT * TO], f32, tag="cs", name="cs")
                tot = lay.tile([128, DT], f32, tag="tot", name="tot")
                for dtt in range(DT):
                    nc.vector.tensor_tensor_scan(
                        out=cs[:, dtt * TO : (dtt + 1) * TO],
                        data0=xA[:, dtt * TO : (dtt + 1) * TO],
                        data1=xA[:, dtt * TO : (dtt + 1) * TO],
                        initial=0.0,
                        op0=Alu.add,
                        op1=Alu.bypass,
                    )
                    nc.vector.tensor_copy(
                        tot[:, dtt : dtt + 1], cs[:, dtt * TO + TO - 1 : dtt * TO + TO]
                    )
                nc.sync.dma_start(bs_in[:], tot[:])
                nc.gpsimd.collective_compute(
                    "AllGather",
                    mybir.AluOpType.bypass,
                    replica_groups=RG,
                    ins=[bs_in[:]],
                    outs=[bs_out[:]],
                )
                blks = lay.tile([128, 4 * DT], f32, tag="blks", name="blks")
                nc.sync.dma_start(
                    blks[:].rearrange("p (r k) -> p r k", r=4),
                    bs_out[:].rearrange("(r p) k -> p r k", p=128),
                )
                pref = lay.tile([128, DT], f32, tag="pref", name="pref")
                nc.vector.tensor_scalar(
                    pref[:], blks[:, 0:DT], cst[:, 24:25], None, Alu.mult
                )
                for r in range(1, 4):
                    nc.vector.scalar_tensor_tensor(
                        out=pref[:],
                        in0=blks[:, r * DT : (r + 1) * DT],
                        scalar=cst[:, 24 + r : 25 + r],
                        in1=pref[:],
                        op0=Alu.mult,
                        op1=Alu.add,
                    )
                # mixed = xA + (cs + pref) * rc
                for dtt in range(DT):
                    nc.vector.scalar_tensor_tensor(
                        out=cs[:, dtt * TO : (dtt + 1) * TO],
                        in0=cs[:, dtt * TO : (dtt + 1) * TO],
                        scalar=pref[:, dtt : dtt + 1],
                        in1=rc_bc[:],
                        op0=Alu.add,
                        op1=Alu.mult,
                    )
                    nc.gpsimd.tensor_tensor(
                        out=mixed[:, dtt * TO : (dtt + 1) * TO],
                        in0=xA[:, dtt * TO : (dtt + 1) * TO],
                        in1=cs[:, dtt * TO : (dtt + 1) * TO],
                        op=Alu.add,
                    )

                dw_sb = wgt.tile([128, DT * RANK], bf, tag="dw", name="dw")
                for kt in range(DT):
                    nc.sync.dma_start(
                        dw_sb[:, kt * RANK : (kt + 1) * RANK],
                        d_dwT[k, kt * 128 : (kt + 1) * 128, :],
                    )
                uw_sb = wgt.tile([128, D], bf, tag="uw", name="uw")
                nc.sync.dma_start(uw_sb[0:RANK, :], d_uwT[k])

                hsb = lay.tile([128, TO], bf, tag="hsb", name="hsb")
                tgf = t_hb()
                ubx = t_zb()
                nc.scalar.activation(ubx[:], mixed[:], Act.Copy)
                sq = t_sq()
                nc.scalar.activation(sq[:], ubx[:], Act.Square)
                pn = py_tile()
                for kt in range(DT):
                    nc.tensor.matmul(
                        pn[0:1, :],
                        lhsT=ones_bf[:],
                        rhs=sq[:, kt * TO : (kt + 1) * TO],
                        start=(kt == 0),
                        stop=(kt == DT - 1),
                    )
                srow = t_srow()
                nc.scalar.activation(
                    srow[:], pn[0:1, :], Act.Abs_reciprocal_sqrt,
                    bias=epsc[0:1, :], scale=1.0 / D,
                )
                sbc = t_sbc()
                nc.gpsimd.partition_broadcast(sbc[:], srow[0:1, :])
                nb = t_ub()
                for dtt in range(DT):
                    nc.vector.tensor_tensor(
                        out=nb[:, dtt * TO : (dtt + 1) * TO],
                        in0=ubx[:, dtt * TO : (dtt + 1) * TO],
                        in1=sbc[:],
                        op=Alu.mult,
                    )
                psh = pg_tile()
                for kt in range(DT):
                    nc.tensor.matmul(
                        psh[0:RANK, 0:TO],
                        lhsT=dw_sb[:, kt * RANK : (kt + 1) * RANK],
                        rhs=nb[:, kt * TO : (kt + 1) * TO],
                        start=(kt == 0),
                        stop=(kt == DT - 1),
                    )
                nc.scalar.activation(
                    hsb[0:RANK, :],
                    psh[0:RANK, 0:TO],
                    Act.Silu,
                    bias=sgc[0:RANK, k : k + 1],
                )
                for md in range(DT):
                    py = py_tile()
                    nc.tensor.matmul(
                        py[:],
                        lhsT=uw_sb[0:RANK, md * 128 : (md + 1) * 128],
                        rhs=hsb[0:RANK, :],
                        start=True,
                        stop=True,
                    )
                    nc.scalar.activation(
                        tgf[:, md * TO : (md + 1) * TO],
                        py[:],
                        Act.Tanh,
                        scale=0.5,
                        bias=sgc[:, 4 + k * 4 + md : 5 + k * 4 + md],
                    )
                # blend: xA = old + (0.5 + 0.5*t) * (mixed - old)
                dfs = lay.tile([128, DT * TO], bf, tag="dfs", name="dfs", bufs=1)
                nc.vector.tensor_tensor(
                    out=dfs[:], in0=mixed[:], in1=old_tile[:], op=Alu.subtract
                )
                nc.vector.tensor_scalar(tgf[:], tgf[:], 0.5, 0.5, Alu.mult, Alu.add)
                nc.gpsimd.tensor_tensor(out=dfs[:], in0=tgf[:], in1=dfs[:], op=Alu.mult)
                nc.vector.tensor_tensor(
                    out=xA[:], in0=old_tile[:], in1=dfs[:], op=Alu.add
                )

            # ---------------- run the model ----------------
            gated2 = None
            for si in range(N_STACKS):
                run_stack([0, 1, 2][si])
                if si == 0:
                    boundary(0, initial)
                elif si == 1:
                    boundary(1, initial)
                    gated2 = keep_tile()  # initial dead
                    nc.vector.tensor_scalar(gated2[:], xA[:], 1.0, None, Alu.mult)
                elif si == 2:
                    boundary(2, gated2)

            if DEBUG_TRUNK_OUT:
                nc.sync.dma_start(d_trunk[:], xA[:])

            # ---------------- final rmsnorm + AG + tied head ----------------
            if not SKIP_HEAD:
                hd = ctx.enter_context(tc.tile_pool(name="hd", bufs=1))
                ob_own = hd.tile([128, DT * TO], bf, tag="obo", name="obo")
                sq = t_sq()
                nc.scalar.activation(sq[:], xA[:], Act.Square)
                pn = py_tile()
                for kt in range(DT):
                    nc.tensor.matmul(
                        pn[0:1, :],
                        lhsT=ones_bf[:],
                        rhs=sq[:, kt * TO : (kt + 1) * TO],
                        start=(kt == 0),
                        stop=(kt == DT - 1),
                    )
                srow = t_srow()
                nc.scalar.activation(
                    srow[:], pn[0:1, :], Act.Abs_reciprocal_sqrt,
                    bias=epsc[0:1, :], scale=1.0 / D,
                )
                sbc = t_sbc()
                nc.gpsimd.partition_broadcast(sbc[:], srow[0:1, :])
                for dtt in range(DT):
                    nc.vector.tensor_tensor(
                        out=ob_own[:, dtt * TO : (dtt + 1) * TO],
                        in0=xA[:, dtt * TO : (dtt + 1) * TO],
                        in1=sbc[:],
                        op=Alu.mult,
                    )
                agi, ago = agiT[256], agoT[256]
                nc.sync.dma_start(agi[:], ob_own[:])
                nc.gpsimd.collective_compute(
                    "AllGather",
                    mybir.AluOpType.bypass,
                    replica_groups=RG,
                    ins=[agi[:]],
                    outs=[ago[0 : 4 * 128, :]],
                )
                ob = hd.tile([128, DT * T], bf, tag="obf", name="obf")
                ob3 = ob[:].rearrange("p (k t) -> p k t", k=DT)
                for r in range(4):
                    nc.sync.dma_start(
                        ob3[:, :, r * TO : (r + 1) * TO],
                        ago[r * 128 : (r + 1) * 128, :].rearrange(
                            "p (k t) -> p k t", k=DT
                        ),
                    )
                for nv in range(NV):
                    nw = min(512, VS - nv * 512)
                    rhsb = hd.tile([128, DT * 512], bf, tag="rhsb", name="rhsb", bufs=3)
                    nc.sync.dma_start(rhsb[:], d_embT[nv])
                    pcb = pc_tile()
                    for mt in range(8):
                        r = mt % 4
                        if r == 0:
                            psl = pg_tile()
                        elif r == 1:
                            psl = pu_tile()
                        else:
                            psl = pcb[:, 0:512] if r == 2 else pcb[:, 512:1024]
                        for kt in range(DT):
                            nc.tensor.matmul(
                                psl[:, :nw],
                                lhsT=ob[:, kt * T + mt * 128 : kt * T + (mt + 1) * 128],
                                rhs=rhsb[:, kt * 512 : kt * 512 + nw],
                                start=(kt == 0),
                                stop=(kt == DT - 1),
                            )
                        lsb = hd.tile([128, 512], f32, tag="lsb", name="lsb", bufs=4)
                        if mt % 2 == 0:
                            nc.scalar.activation(lsb[:, :nw], psl[:, :nw], Act.Copy)
                        else:
                            nc.vector.tensor_copy(lsb[:, :nw], psl[:, :nw])
                        outq = nc.gpsimd if mt % 2 == 0 else nc.sync
                        outq.dma_start(
                            d_out[mt * 128 : (mt + 1) * 128, nv * 512 : nv * 512 + nw],
                            lsb[:, :nw],
                        )

    nc.finalize()
    _prog_cache[key] = nc
    return nc


def prep_inputs(inputs):
    """Host-side: full model inputs -> list of 8 per-core in_maps."""
    idx = np.asarray(inputs["idx"])
    emb = _f32(inputs["emb"])
    pos = _f32(inputs["pos"])[0, :T]  # [T, D]
    we = _f32(inputs["emb_norm_w"])
    ts = _f32(inputs["token_shift"])
    mgw = _f32(inputs["mem_gate_w"])
    mgb = _f32(inputs["mem_gate_b"])
    memp = _f32(inputs["memory_p"])
    fnw = _f32(inputs["final_norm_w"])

    mgwT = np.ascontiguousarray(mgw.T)

    stack_in = {}
    for s in ("up", "dn"):
        nw = _f32(inputs[f"{s}_norm_w"])  # [NL, D]
        cw = _f32(inputs[f"{s}_conv_w"])[:, :, 0, :]  # [NL, D, K]
        cb = _f32(inputs[f"{s}_conv_b"])  # [NL, D]
        w1 = _f32(inputs[f"{s}_w1"])
        w2 = _f32(inputs[f"{s}_w2"])
        w3 = _f32(inputs[f"{s}_w3"])
        kp = _f32(inputs[f"{s}_kp"])
        ki = _f32(inputs[f"{s}_ki"])
        gn = _f32(inputs[f"{s}_gnorm"])
        cwb = np.zeros((NL, 128, DT), np.float32)
        cwt = np.zeros((NL, 128, KK), np.float32)
        cdiag = np.zeros((NL, 128, DT * KK * 128), np.float32)
        gdiag = np.zeros((NL, 128, DT * 128), np.float32)
        for li in range(NL):
            # conv path: gnorm cancels inside the double rmsnorm; fold norm_w
            # only. gnorm survives only in the residual base (gdiag).
            gfold = gn[li - 1] if li > 0 else np.ones(D, np.float32)
            cwf = cw[li] * nw[li][:, None]  # [D, K]
            taps = cwf[:, ::-1]  # tap m multiplies shift m*d
            cwb[li] = _cols(cb[li])
            cwt[li] = taps[3 * 128 : 4 * 128, :]
            for dtt in range(DT):
                for m in range(KK):
                    cdiag[li, :, (dtt * KK + m) * 128 : (dtt * KK + m + 1) * 128] = np.diag(
                        taps[dtt * 128 : (dtt + 1) * 128, m]
                    )
                gdiag[li, :, dtt * 128 : (dtt + 1) * 128] = np.diag(
                    gfold[dtt * 128 : (dtt + 1) * 128]
                )
        pid = np.zeros((128, (NL - 1) * 12), np.float32)
        for li in range(1, NL):
            pb = (li - 1) * 12
            pid[:, pb : pb + 4] = _cols(kp[li - 1] + ki[li - 1] / li)
            pid[:, pb + 4 : pb + 8] = _cols(ki[li - 1] / li)
        stack_in[f"{s}_cwb"] = np.ascontiguousarray(cwb)
        stack_in[f"{s}_cwt"] = np.ascontiguousarray(cwt)
        stack_in[f"{s}_pid"] = pid
        stack_in[f"{s}_cdiag"] = np.ascontiguousarray(cdiag).astype(BF16)
        stack_in[f"{s}_gdiag"] = np.ascontiguousarray(gdiag).astype(BF16)
        w13 = np.concatenate([w1.transpose(0, 2, 1), w3.transpose(0, 2, 1)], axis=2)
        w13p = np.ascontiguousarray(
            w13.reshape(NL, DT, 128, 2 * HID).transpose(0, 2, 1, 3).reshape(NL, 128, DT * 2 * HID)
        )
        stack_in[f"{s}_w13p"] = w13p.astype(BF16)
        w2T = w2.transpose(0, 2, 1)  # [NL, HID, D]
        w2p = np.ascontiguousarray(
            w2T.reshape(NL, HT, 128, D).transpose(0, 2, 1, 3).reshape(NL, 128, HT * D)
        )
        stack_in[f"{s}_w2p"] = w2p.astype(BF16)

    sgn = _f32(inputs["sg_norm"])
    sgdw = _f32(inputs["sg_down_w"])
    sgdb = _f32(inputs["sg_down_b"])
    sguw = _f32(inputs["sg_up_w"])
    sgub = _f32(inputs["sg_up_b"])
    dwT = np.stack(
        [np.ascontiguousarray(sgdw[k].T * sgn[k][:, None]) for k in range(3)]
    ).astype(BF16)
    uwT = np.stack([np.ascontiguousarray(sguw[k].T) for k in range(3)]).astype(BF16)
    sgc = np.zeros((128, 16), np.float32)
    for k in range(3):
        sgc[0:RANK, k] = sgdb[k]
        sgc[:, 4 + k * 4 : 8 + k * 4] = _cols(0.5 * sgub[k])

    embT = np.ascontiguousarray((emb.T * fnw[:, None]))  # [D, V] f32

    common = dict(
        mgwT=mgwT,
        emb_tbl=emb,
        sg_dwT=dwT,
        sg_uwT=uwT,
        sg_cols=sgc,
        **stack_in,
    )

    in_maps = []
    for c in range(NCORES):
        b = c // 4
        q = c % 4
        beta = q * TO
        m = dict(common)
        # idx: 3 chunks of 128 tokens: [beta-128, beta+256); q=0 pads with idx[0]
        tok = np.arange(beta - 128, beta + TO)
        tokc = np.clip(tok, 0, T - 1)
        m["idx_rs"] = np.ascontiguousarray(
            idx[b][tokc].astype(np.int32).reshape(3, 128).T
        )
        m["pos_rs"] = np.ascontiguousarray(
            pos[tokc].reshape(3, 128, D).transpose(1, 0, 2).reshape(128, 3 * D)
        )
        cst = np.zeros((128, 64), np.float32)
        # col0 coefs: q=0 -> shifted[0]=x[0]: cself0=we, cprev0=0
        if q == 0:
            cst[:, 0:4] = _cols(we)
            cst[:, 12:16] = 0.0
        else:
            cst[:, 0:4] = _cols((1.0 - ts) * we)
            cst[:, 12:16] = _cols(ts * we)
        cst[:, 4:8] = _cols(ts * we)
        cst[:, 8:12] = _cols((1.0 - ts) * we)
        cst[:, 16:20] = _cols(0.5 * mgb)
        cst[:, 20:24] = _cols(memp[b])
        for r in range(4):
            cst[:, 24 + r] = 1.0 if r < q else 0.0
        m["cst"] = cst
        rc = (MIX_W / (np.arange(beta + 1, beta + TO + 1, dtype=np.float32)))[None, :]
        m["rc_bc"] = np.ascontiguousarray(np.broadcast_to(rc, (128, TO)))
        # halo row-offset tables: rows into ago [640, ...]; block 4 is zeros
        offs = np.zeros((128, 2), np.int32)
        p = np.arange(128, dtype=np.int32)
        offs[:, 0] = (q - 1) * 128 + p if q >= 1 else 4 * 128 + p
        offs[:, 1] = (q - 2) * 128 + p if q >= 2 else 4 * 128 + p
        m["halo_offs"] = offs
        # head vocab shard
        esh = embT[:, q * VS : (q + 1) * VS]  # [D, VS]
        eshpad = np.zeros((D, NV * 512), np.float32)
        eshpad[:, :VS] = esh
        eshp = np.ascontiguousarray(
            eshpad.reshape(DT, 128, NV, 512).transpose(2, 1, 0, 3).reshape(NV, 128, DT * 512)
        )
        m["embT_sh"] = eshp.astype(BF16)
        in_maps.append(m)
    return in_maps


LAST_RESULTS = None


def kernel(**inputs):
    global LAST_RESULTS
    from concourse.bass_utils import run_bass_kernel_spmd

    nc = build_program()
    in_maps = prep_inputs(inputs)
    trace = bool(int(os.environ.get("KB_TRACE", "0")))
    res = run_bass_kernel_spmd(nc, in_maps, core_ids=list(range(NCORES)), trace=trace)
    LAST_RESULTS = res
    out = np.zeros((B, T, VOCAB), np.float32)
    for c in range(NCORES):
        b = c // 4
        q = c % 4
        out[b, :, q * VS : (q + 1) * VS] = res.results[c]["logits_sh"]
    return out


# revision 19
# speedup vs baseline: 1.0051x; 1.0051x over previous
"""Trainium2 Bass kernel for nn_MemPIDModel (dense_cnn) — sequence-parallel.

Strategy (8 NeuronCores):
  - core c handles sample b = c//4, token quarter q = c%4 (256 tokens each)
  - trunk is sequence-parallel: per layer, each core computes its 256-token
    slice; the causal dilated conv's halo is exchanged via a 4-rank
    AllGather of the normed conv input (bf16), sized to the layer's actual
    receptive field h = min(14*dil, 256) tokens
  - halo readback: indirect DMA with per-core row-offset tables (an
    always-zero block in each AG buffer provides causal zero padding, so
    one SPMD program works for all cores); full-row gathers land strided
    across the 4 d-tiles in one instruction
  - conv runs fully on PE as 15 diag-matmul accumulations per d-tile at
    256-col streams (matmul issue cadence bound -> widest possible)
  - mix boundaries: local cumsum scan + tiny AllGather of slice totals for
    the cross-core prefix (mask-weighted sum keeps the program uniform)
  - head: AllGather of the normed trunk output, then each core computes its
    vocab shard v = c%4 over all 1024 tokens ([1024,512]@[512,8000])
"""

import os
import sys
import numpy as np

sys.path.insert(0, "/opt/trn_rl_repo")

import ml_dtypes

B = 2
T = 1024
D = 512
HID = 1024
KK = 15
VOCAB = 32000
RANK = 64
NL = 6
MIX_W = 0.1
UP_DIL = [1, 2, 4, 8, 16, 32]
DN_DIL = UP_DIL[::-1]
EPS = 1e-6
NCORES = 8
VSHARDS = 4
VS = VOCAB // VSHARDS  # 8000
DT = D // 128  # 4 D-tiles
HT = HID // 128  # 8 H-tiles
TO = 256  # own tokens per core
POFF = 512  # own region offset inside xnb (per d-tile)
CONVW = POFF + TO  # 768 padded conv input width per d-tile
NV = (VS + 511) // 512  # 16 head column chunks (last is 320 wide)
HS = sorted({min(14 * d, 256) for d in UP_DIL})  # AG payload widths

BF16 = ml_dtypes.bfloat16

# debug knobs (affect program shape; kernel cache key includes them)
N_LAYERS = int(os.environ.get("KB_LAYERS", str(NL)))
N_STACKS = int(os.environ.get("KB_STACKS", "3"))
SKIP_HEAD = bool(int(os.environ.get("KB_SKIP_HEAD", "0")))
DEBUG_TRUNK_OUT = bool(int(os.environ.get("KB_TRUNK_OUT", "0")))

_prog_cache = {}


def _f32(x):
    return np.ascontiguousarray(np.asarray(x), dtype=np.float32)


def _cols(v):
    """[D] vector -> [128, DT] A-layout per-partition columns."""
    return np.ascontiguousarray(_f32(v).reshape(DT, 128).T)


def _stack_dils(stack_idx):
    return UP_DIL if stack_idx in (0, 2) else DN_DIL


def build_program():
    key = (N_LAYERS, N_STACKS, SKIP_HEAD, DEBUG_TRUNK_OUT)
    if key in _prog_cache:
        return _prog_cache[key]

    import concourse.bass as bass
    import concourse.mybir as mybir
    import concourse.tile as tile
    from concourse import bacc
    from concourse.masks import make_identity

    dt = mybir.dt
    Alu = mybir.AluOpType
    Act = mybir.ActivationFunctionType

    nc = bacc.Bacc(None, target_bir_lowering=False, debug=False)

    RG = [[0, 1, 2, 3], [4, 5, 6, 7]]

    # ---------------- DRAM I/O ----------------
    d_idx = nc.dram_tensor("idx_rs", [128, 3], dt.int32, kind="ExternalInput")
    d_emb = nc.dram_tensor("emb_tbl", [VOCAB, D], dt.float32, kind="ExternalInput")
    d_pos = nc.dram_tensor("pos_rs", [128, 3 * D], dt.float32, kind="ExternalInput")
    d_cst = nc.dram_tensor("cst", [128, 64], dt.float32, kind="ExternalInput")
    d_rc = nc.dram_tensor("rc_bc", [128, TO], dt.float32, kind="ExternalInput")
    d_mgw = nc.dram_tensor("mgwT", [D, D], dt.float32, kind="ExternalInput")
    d_offs = nc.dram_tensor("halo_offs", [128, 2], dt.int32, kind="ExternalInput")

    d_cwb = {}
    d_cwt = {}
    d_w13 = {}
    d_w2 = {}
    d_pid = {}
    d_cdiag = {}
    d_gdiag = {}
    for s in ("up", "dn"):
        d_cwb[s] = nc.dram_tensor(
            f"{s}_cwb", [NL, 128, DT], dt.float32, kind="ExternalInput"
        )
        d_cwt[s] = nc.dram_tensor(
            f"{s}_cwt", [NL, 128, KK], dt.float32, kind="ExternalInput"
        )
        d_w13[s] = nc.dram_tensor(
            f"{s}_w13p", [NL, 128, DT * 2 * HID], dt.bfloat16, kind="ExternalInput"
        )
        d_w2[s] = nc.dram_tensor(
            f"{s}_w2p", [NL, 128, HT * D], dt.bfloat16, kind="ExternalInput"
        )
        d_pid[s] = nc.dram_tensor(
            f"{s}_pid", [128, (NL - 1) * 12], dt.float32, kind="ExternalInput"
        )
        d_cdiag[s] = nc.dram_tensor(
            f"{s}_cdiag", [NL, 128, DT * KK * 128], dt.bfloat16, kind="ExternalInput"
        )
        d_gdiag[s] = nc.dram_tensor(
            f"{s}_gdiag", [NL, 128, DT * 128], dt.bfloat16, kind="ExternalInput"
        )
    d_dwT = nc.dram_tensor("sg_dwT", [3, D, RANK], dt.bfloat16, kind="ExternalInput")
    d_uwT = nc.dram_tensor("sg_uwT", [3, RANK, D], dt.bfloat16, kind="ExternalInput")
    d_sgc = nc.dram_tensor("sg_cols", [128, 16], dt.float32, kind="ExternalInput")
    d_embT = nc.dram_tensor("embT_sh", [NV, 128, DT * 512], dt.bfloat16, kind="ExternalInput")

    d_out = nc.dram_tensor("logits_sh", [T, VS], dt.float32, kind="ExternalOutput")
    if DEBUG_TRUNK_OUT:
        d_trunk = nc.dram_tensor("trunk_out", [128, DT * TO], dt.float32, kind="ExternalOutput")

    f32 = dt.float32
    bf = dt.bfloat16

    with tile.TileContext(nc) as tc:
        import contextlib

        ctx = contextlib.ExitStack()
        with ctx:
            const = ctx.enter_context(tc.tile_pool(name="const", bufs=1))
            master = ctx.enter_context(tc.tile_pool(name="master", bufs=1))
            lay = ctx.enter_context(tc.tile_pool(name="lay", bufs=1))
            wgt = ctx.enter_context(tc.tile_pool(name="wgt", bufs=2))
            psum = ctx.enter_context(tc.tile_pool(name="psum", bufs=1, space="PSUM"))
            dram = ctx.enter_context(tc.tile_pool(name="dram", bufs=1, space="DRAM"))

            # ---------------- constants ----------------
            epsc = const.tile([128, 1], f32, tag="epsc")
            nc.vector.memset(epsc[:], EPS)
            ones_bf = const.tile([128, 1], bf, tag="ones")
            nc.vector.memset(ones_bf[:], 1.0)
            ones_row = const.tile([1, 128], bf, tag="onesr")
            nc.vector.memset(ones_row[:], 1.0)
            ident = const.tile([128, 128], f32, tag="ident")
            make_identity(nc, ident[:])
            cst = const.tile([128, 64], f32, tag="cst")
            nc.sync.dma_start(cst[:], d_cst[:])
            rc_bc = const.tile([128, TO], f32, tag="rc")
            nc.sync.dma_start(rc_bc[:], d_rc[:])
            sgc = const.tile([128, 16], f32, tag="sgc")
            nc.sync.dma_start(sgc[:], d_sgc[:])
            offs = const.tile([128, 2], dt.int32, tag="offs")
            nc.sync.dma_start(offs[:], d_offs[:])
            pidc = {}
            for s in ("up", "dn"):
                pidc[s] = const.tile(
                    [128, (NL - 1) * 12], f32, tag=f"pid_{s}", name=f"pid_{s}"
                )
                nc.sync.dma_start(pidc[s][:], d_pid[s][:])

            # AllGather buffers per payload width h: out rows [0:512] written
            # by AG, rows [512:640] stay zero (causal pad source)
            zero_sb = const.tile([128, DT * TO], bf, tag="zsb")
            nc.gpsimd.memset(zero_sb[:], 0.0)
            agiT = {}
            agoT = {}
            for h in HS:
                agiT[h] = dram.tile([128, DT * h], bf, tag=f"agi{h}", name=f"agi{h}")
                agoT[h] = dram.tile([5 * 128, DT * h], bf, tag=f"ago{h}", name=f"ago{h}")
                nc.sync.dma_start(agoT[h][4 * 128 : 5 * 128, :], zero_sb[:, 0 : DT * h])
            bs_in = dram.tile([128, DT], f32, tag="bs_in", bufs=1)
            bs_out = dram.tile([4 * 128, DT], f32, tag="bs_out", bufs=1)

            # persistent activations (A-layout, free index = dt*TO + t)
            xA = master.tile([128, DT * TO], f32, tag="xA")

            def keep_tile():  # initial, then gated2 (sequential lifetimes)
                return master.tile([128, DT * TO], f32, tag="keep", name="keep")

            def f32a_tile():  # integ during stacks / mixed during boundaries
                return lay.tile([128, DT * TO], f32, tag="f32a", name="f32a")

            def t_zb():
                return lay.tile([128, DT * TO], bf, tag="zb", name="zb", bufs=2)

            def t_ub():
                return lay.tile([128, DT * TO], bf, tag="ub", name="ub", bufs=2)

            def t_sq():
                return lay.tile([128, DT * TO], bf, tag="sq", name="sq", bufs=2)

            def t_hb():
                return lay.tile([128, DT * TO], bf, tag="hb", name="hb", bufs=2)

            def t_sbc():
                return lay.tile([128, TO], bf, tag="sbc", name="sbc", bufs=2)

            def t_srow():
                return lay.tile([1, TO], bf, tag="srow", name="srow", bufs=2)

            def t_pch():
                return lay.tile([128, HT * TO], bf, tag="pch", name="pch", bufs=2)

            def t_gst():
                return lay.tile([128, 2 * TO], bf, tag="gst", name="gst", bufs=2)

            def t_xnb():
                return lay.tile([128, DT * CONVW], bf, tag="xnb", name="xnb", bufs=2)

            # PSUM: pc [128,1024]=2 banks, pg/pu/py [*,<=512] bufs=2
            def pc_tile():
                return psum.tile([128, DT * TO], f32, tag="pc", bufs=1, name="pc")

            def pg_tile():
                return psum.tile([128, 2 * TO], f32, tag="pg", bufs=2, name="pg")

            def pu_tile():
                return psum.tile([128, 2 * TO], f32, tag="pu", bufs=2, name="pu")

            def py_tile():
                return psum.tile([128, TO], f32, tag="py", bufs=2, name="py")

            # ---------------- P0: gather + embnorm + shift + mem ----------------
            with tc.tile_pool(name="p0", bufs=1) as p0:
                idx_sb = p0.tile([128, 3], dt.int32, tag="idx")
                nc.sync.dma_start(idx_sb[:], d_idx[:])
                gth = p0.tile([128, 3 * D], f32, tag="gth")
                for c in range(3):
                    nc.gpsimd.indirect_dma_start(
                        out=gth[:, c * D : (c + 1) * D],
                        out_offset=None,
                        in_=d_emb[:],
                        in_offset=bass.IndirectOffsetOnAxis(ap=idx_sb[:, c : c + 1], axis=0),
                    )
                pos_sb = p0.tile([128, 3 * D], f32, tag="pos")
                nc.sync.dma_start(pos_sb[:], d_pos[:])
                nc.vector.tensor_tensor(
                    out=gth[:], in0=gth[:], in1=pos_sb[:], op=Alu.add
                )
                ss = p0.tile([128, 3], f32, tag="ss")
                sqt = p0.tile([128, D], f32, tag="sqt")
                for c in range(3):
                    nc.scalar.activation(
                        sqt[:],
                        gth[:, c * D : (c + 1) * D],
                        Act.Square,
                        accum_out=ss[:, c : c + 1],
                    )
                nc.scalar.activation(ss[:], ss[:], Act.Ln, bias=epsc[:], scale=1.0 / D)
                nc.scalar.activation(ss[:], ss[:], Act.Exp, scale=-0.5)
                for c in range(3):
                    nc.vector.tensor_scalar(
                        gth[:, c * D : (c + 1) * D],
                        gth[:, c * D : (c + 1) * D],
                        ss[:, c : c + 1],
                        None,
                        Alu.mult,
                    )
                # transpose token-major -> A-layout x_n [128, DT*384]
                x_n = p0.tile([128, DT * 384], f32, tag="xn")
                for c in range(3):
                    pst = pg_tile()
                    for dtt in range(2):
                        nc.tensor.transpose(
                            out=pst[:, dtt * 128 : (dtt + 1) * 128],
                            in_=gth[:, c * D + dtt * 128 : c * D + (dtt + 1) * 128],
                            identity=ident[:],
                        )
                    pst2 = pu_tile()
                    for dtt in range(2):
                        nc.tensor.transpose(
                            out=pst2[:, dtt * 128 : (dtt + 1) * 128],
                            in_=gth[:, c * D + (2 + dtt) * 128 : c * D + (3 + dtt) * 128],
                            identity=ident[:],
                        )
                    for dtt in range(2):
                        nc.vector.tensor_copy(
                            x_n[:, dtt * 384 + c * 128 : dtt * 384 + (c + 1) * 128],
                            pst[:, dtt * 128 : (dtt + 1) * 128],
                        )
                        nc.vector.tensor_copy(
                            x_n[:, (2 + dtt) * 384 + c * 128 : (2 + dtt) * 384 + (c + 1) * 128],
                            pst2[:, dtt * 128 : (dtt + 1) * 128],
                        )
                # mem gate: mem = sigmoid(memp @ mgw.T + mgb)
                ps_mem = py_tile()
                mgw_sb = p0.tile([128, DT * D], f32, tag="mgw", name="mgw_sb")
                nc.scalar.dma_start(
                    mgw_sb[:].rearrange("p (k d) -> p k d", k=DT),
                    d_mgw[:].rearrange("(k p) d -> p k d", p=128),
                )
                for kt in range(DT):
                    for m in range(DT):
                        nc.tensor.matmul(
                            ps_mem[:, m : m + 1],
                            lhsT=mgw_sb[:, kt * D + m * 128 : kt * D + (m + 1) * 128],
                            rhs=cst[:, 20 + kt : 21 + kt],
                            start=(kt == 0),
                            stop=(kt == DT - 1),
                        )
                tmem = p0.tile([128, 4], f32, tag="tmem")
                for m in range(DT):
                    nc.scalar.activation(
                        tmem[:, m : m + 1],
                        ps_mem[:, m : m + 1],
                        Act.Tanh,
                        scale=0.5,
                        bias=cst[:, 16 + m : 17 + m],
                    )
                nc.vector.tensor_scalar(tmem[:], tmem[:], 0.5, 0.5, Alu.mult, Alu.add)
                # token shift + mem (own tokens live at x_n cols 128..384)
                tsh = p0.tile([128, TO], f32, tag="tsh")
                for dtt in range(DT):
                    ox = dtt * 384 + 128  # own region in x_n
                    oa = dtt * TO  # xA
                    nc.vector.tensor_scalar(
                        tsh[:, 0:1], x_n[:, ox - 1 : ox], cst[:, 12 + dtt : 13 + dtt],
                        None, Alu.mult,
                    )
                    nc.vector.scalar_tensor_tensor(
                        out=xA[:, oa : oa + 1],
                        in0=x_n[:, ox : ox + 1],
                        scalar=cst[:, dtt : dtt + 1],
                        in1=tsh[:, 0:1],
                        op0=Alu.mult,
                        op1=Alu.add,
                    )
                    nc.vector.tensor_scalar(
                        tsh[:, 1:TO],
                        x_n[:, ox + 1 : ox + TO],
                        cst[:, 8 + dtt : 9 + dtt],
                        None,
                        Alu.mult,
                    )
                    nc.vector.scalar_tensor_tensor(
                        out=xA[:, oa + 1 : oa + TO],
                        in0=x_n[:, ox : ox + TO - 1],
                        scalar=cst[:, 4 + dtt : 5 + dtt],
                        in1=tsh[:, 1:TO],
                        op0=Alu.mult,
                        op1=Alu.add,
                    )
                    nc.vector.tensor_scalar(
                        xA[:, oa : oa + TO], xA[:, oa : oa + TO],
                        tmem[:, dtt : dtt + 1], None, Alu.add,
                    )
            initial = keep_tile()
            nc.vector.tensor_scalar(initial[:], xA[:], 1.0, None, Alu.mult)

            # ---------------- conv block stack ----------------
            def load_layer_weights(s, li, q2=None):
                w = {}
                if q2 is None:
                    q2 = nc.scalar
                w["cwb"] = wgt.tile([128, DT], f32, tag="cwb", name="cwb")
                nc.sync.dma_start(w["cwb"][:], d_cwb[s][li])
                w["cwt"] = wgt.tile([128, KK], f32, tag="cwt", name="cwt")
                nc.sync.dma_start(w["cwt"][:], d_cwt[s][li])
                w["w13"] = wgt.tile([128, DT * 2 * HID], bf, tag="w13", name="w13")
                for qq in range(2):
                    qs = qq * 4 * HID
                    nc.sync.dma_start(
                        w["w13"][:, qs : qs + 4 * HID], d_w13[s][li, :, qs : qs + 4 * HID]
                    )
                w["w2"] = wgt.tile([128, HT * D], bf, tag="w2", name="w2")
                q2.dma_start(w["w2"][:], d_w2[s][li])
                w["cdiag"] = wgt.tile([128, DT * KK * 128], bf, tag="cdiag", name="cdiag")
                q2.dma_start(w["cdiag"][:], d_cdiag[s][li])
                w["gdiag"] = wgt.tile([128, DT * 128], bf, tag="gdiag", name="gdiag")
                q2.dma_start(w["gdiag"][:], d_gdiag[s][li])
                return w

            def run_stack(stack_idx):
                s = "up" if stack_idx in (0, 2) else "dn"
                dils = _stack_dils(stack_idx)
                integ = f32a_tile()
                nc.vector.tensor_scalar(integ[:], xA[:], 1.0, None, Alu.mult)

                wts = {}
                for li in range(min(2, N_LAYERS)):
                    wts[li] = load_layer_weights(s, li, q2=nc.sync)

                st = {}

                def S1(li):
                    # rmsnorm scale -> xnb own region (ub/sq made by S3 tail
                    # of the previous layer for li>0)
                    cs = st.setdefault(li, {})
                    if li == 0:
                        ub = cs["ub"] = t_ub()
                        nc.scalar.activation(ub[:], xA[:], Act.Copy)
                        sq = cs["sq"] = t_sq()
                        nc.scalar.activation(sq[:], ub[:], Act.Square)
                    ub = cs["ub"]
                    sq = cs["sq"]
                    pn = py_tile()
                    for kt in range(DT):
                        nc.tensor.matmul(
                            pn[0:1, :],
                            lhsT=ones_bf[:],
                            rhs=sq[:, kt * TO : (kt + 1) * TO],
                            start=(kt == 0),
                            stop=(kt == DT - 1),
                        )
                    srow = t_srow()
                    nc.scalar.activation(
                        srow[:], pn[0:1, :], Act.Abs_reciprocal_sqrt,
                        bias=epsc[0:1, :], scale=1.0 / D,
                    )
                    # broadcast via 1-row PE matmul (keeps gpsimd queue clear)
                    sbc = py_tile()
                    nc.tensor.matmul(
                        sbc[:], lhsT=ones_row[0:1, :], rhs=srow[0:1, :],
                        start=True, stop=True,
                    )
                    xnb = st[("xnb", li)]
                    for dtt in range(DT):
                        nc.vector.tensor_tensor(
                            out=xnb[:, dtt * CONVW + POFF : dtt * CONVW + POFF + TO],
                            in0=ub[:, dtt * TO : (dtt + 1) * TO],
                            in1=sbc[:],
                            op=Alu.mult,
                        )

                def AG(li):
                    # exchange last h own tokens; read back h-token halo
                    d = dils[li]
                    h = min(14 * d, TO)
                    xnb = st[("xnb", li)]
                    agi, ago = agiT[h], agoT[h]
                    xnb3 = xnb[:].rearrange("p (d w) -> p d w", d=DT)
                    nc.sync.dma_start(agi[:], xnb3[:, :, POFF + TO - h : POFF + TO])
                    nc.gpsimd.collective_compute(
                        "AllGather",
                        mybir.AluOpType.bypass,
                        replica_groups=RG,
                        ins=[agi[:]],
                        outs=[ago[0 : 4 * 128, :]],
                    )
                    # prev1: its last h tokens -> staging -> xnb [POFF-h, POFF)
                    stg = lay.tile([128, DT * TO], bf, tag="stg", name="stg", bufs=2)
                    nc.gpsimd.indirect_dma_start(
                        out=stg[:, 0 : DT * h],
                        out_offset=None,
                        in_=ago[:],
                        in_offset=bass.IndirectOffsetOnAxis(ap=offs[:, 0:1], axis=0),
                        element_offset=0,
                    )
                    stg3 = stg[:, 0 : DT * h].rearrange("p (d w) -> p d w", d=DT)
                    nc.vector.tensor_copy(xnb3[:, :, POFF - h : POFF], stg3[:, :, :])
                    if 14 * d > TO:
                        # d=32: prev2 tokens [beta-448,beta-256) via its cols [64,256)
                        stg2 = lay.tile([128, DT * TO], bf, tag="stg", name="stg2", bufs=2)
                        nc.gpsimd.indirect_dma_start(
                            out=stg2[:, 0 : DT * h],
                            out_offset=None,
                            in_=ago[:],
                            in_offset=bass.IndirectOffsetOnAxis(ap=offs[:, 1:2], axis=0),
                            element_offset=0,
                        )
                        stg23 = stg2[:, 0 : DT * h].rearrange("p (d w) -> p d w", d=DT)
                        nc.vector.tensor_copy(
                            xnb3[:, :, 64:TO], stg23[:, :, 64:TO]
                        )

                def S2(li):
                    # conv: dtiles 0-2 on PE (diag matmuls, own/halo col-split
                    # for d>=16 so own-col work runs during the AllGather);
                    # dtile 3 on Pool as an STT tap chain
                    d = dils[li]
                    cs = st[li]
                    w = wts[li]
                    xnb = st[("xnb", li)]
                    cacc = lay.tile([128, TO], bf, tag="cacc", name="cacc", bufs=2)
                    ob3 = 3 * CONVW

                    def tap_in(m):
                        stt = ob3 + POFF - m * d
                        return xnb[:, stt : stt + TO]

                    nc.vector.tensor_scalar(
                        cacc[:], tap_in(KK - 1), w["cwt"][:, KK - 1 : KK], None, Alu.mult
                    )
                    for m in range(KK - 2, -1, -1):
                        nc.vector.scalar_tensor_tensor(
                            out=cacc[:],
                            in0=tap_in(m),
                            scalar=w["cwt"][:, m : m + 1],
                            in1=cacc[:],
                            op0=Alu.mult,
                            op1=Alu.add,
                        )
                    psc = pc_tile()
                    for dtt in range(3):
                        ob = dtt * CONVW
                        oc = dtt * TO
                        if d >= 16:
                            # own-col parts first (no halo dependency)
                            for m in range(KK):
                                lo = m * d
                                if lo >= TO:
                                    continue
                                nc.tensor.matmul(
                                    psc[:, oc + lo : oc + TO],
                                    lhsT=w["cdiag"][:, (dtt * KK + m) * 128 : (dtt * KK + m + 1) * 128],
                                    rhs=xnb[:, ob + POFF : ob + POFF + TO - lo],
                                    start=(m == 0),
                                    stop=(m == KK - 1),
                                )
                            for m in range(1, KK):
                                lo = min(m * d, TO)
                                nc.tensor.matmul(
                                    psc[:, oc : oc + lo],
                                    lhsT=w["cdiag"][:, (dtt * KK + m) * 128 : (dtt * KK + m + 1) * 128],
                                    rhs=xnb[:, ob + POFF - m * d : ob + POFF - m * d + lo],
                                    start=False,
                                    stop=(m == KK - 1),
                                )
                        else:
                            for m in range(KK - 1, -1, -1):
                                stt = ob + POFF - m * d
                                nc.tensor.matmul(
                                    psc[:, oc : oc + TO],
                                    lhsT=w["cdiag"][:, (dtt * KK + m) * 128 : (dtt * KK + m + 1) * 128],
                                    rhs=xnb[:, stt : stt + TO],
                                    start=(m == KK - 1),
                                    stop=(m == 0),
                                )
                    hb = cs["hb"] = t_hb()
                    for dtt in range(3):
                        nc.scalar.activation(
                            hb[:, dtt * TO : (dtt + 1) * TO],
                            psc[:, dtt * TO : (dtt + 1) * TO],
                            Act.Silu,
                            bias=w["cwb"][:, dtt : dtt + 1],
                        )
                    nc.scalar.activation(
                        hb[:, 3 * TO : 4 * TO],
                        cacc[:],
                        Act.Silu,
                        bias=w["cwb"][:, 3:4],
                    )

                def S3(li):
                    cs = st[li]
                    w = wts[li]
                    xnb = st[("xnb", li)]
                    hb = cs["hb"]
                    pch = t_pch()
                    for pr in range(4):
                        pg = pg_tile()
                        for j in range(2):
                            kh = pr * 2 + j
                            for kt in range(DT):
                                nc.tensor.matmul(
                                    pg[:, j * TO : (j + 1) * TO],
                                    lhsT=w["w13"][:, kt * 2 * HID + kh * 128 : kt * 2 * HID + (kh + 1) * 128],
                                    rhs=hb[:, kt * TO : (kt + 1) * TO],
                                    start=(kt == 0),
                                    stop=(kt == DT - 1),
                                )
                        gst = t_gst()
                        nc.scalar.activation(gst[:], pg[:], Act.Silu)
                        pu = pu_tile()
                        for j in range(2):
                            kh = pr * 2 + j
                            for kt in range(DT):
                                nc.tensor.matmul(
                                    pu[:, j * TO : (j + 1) * TO],
                                    lhsT=w["w13"][:, kt * 2 * HID + HID + kh * 128 : kt * 2 * HID + HID + (kh + 1) * 128],
                                    rhs=hb[:, kt * TO : (kt + 1) * TO],
                                    start=(kt == 0),
                                    stop=(kt == DT - 1),
                                )
                        nc.vector.tensor_tensor(
                            out=pch[:, pr * 2 * TO : (pr + 1) * 2 * TO],
                            in0=gst[:],
                            in1=pu[:],
                            op=Alu.mult,
                        )
                    for md in range(DT):
                        py = py_tile()
                        if li > 0:
                            base = xnb[:, md * CONVW + POFF : md * CONVW + POFF + TO]
                        else:
                            base = cs["ub"][:, md * TO : (md + 1) * TO]
                        nc.tensor.matmul(
                            py[:],
                            lhsT=w["gdiag"][:, md * 128 : (md + 1) * 128],
                            rhs=base,
                            start=True,
                            stop=False,
                        )
                        for kh in range(HT):
                            nc.tensor.matmul(
                                py[:],
                                lhsT=w["w2"][:, kh * D + md * 128 : kh * D + (md + 1) * 128],
                                rhs=pch[:, kh * TO : (kh + 1) * TO],
                                start=False,
                                stop=(kh == HT - 1),
                            )
                        xs = xA[:, md * TO : (md + 1) * TO]
                        if md % 2 == 0:
                            nc.scalar.activation(xs, py[:], Act.Copy)
                        else:
                            nc.vector.tensor_copy(xs, py[:])
                    # tail: per d-tile integ update + next layer's PID/silu/sq
                    if li < N_LAYERS - 1:
                        pc = pidc[s]
                        pb = li * 12
                        zb = t_zb()
                        nxt = st.setdefault(li + 1, {})
                        ub2 = nxt["ub"] = t_ub()
                        sq2 = nxt["sq"] = t_sq()
                        for dtt in range(DT):
                            sl = slice(dtt * TO, (dtt + 1) * TO)
                            # zb = (kp+ki')*xA + ki'*integ_old
                            nc.vector.tensor_scalar(
                                zb[:, sl], xA[:, sl],
                                pc[:, pb + dtt : pb + 1 + dtt], None, Alu.mult,
                            )
                            nc.vector.scalar_tensor_tensor(
                                out=zb[:, sl],
                                in0=integ[:, sl],
                                scalar=pc[:, pb + 4 + dtt : pb + 5 + dtt],
                                in1=zb[:, sl],
                                op0=Alu.mult,
                                op1=Alu.add,
                            )
                            nc.gpsimd.tensor_tensor(
                                out=integ[:, sl], in0=integ[:, sl],
                                in1=xA[:, sl], op=Alu.add,
                            )
                            nc.scalar.activation(
                                ub2[:, sl], zb[:, sl], Act.Silu
                            )
                            nc.scalar.activation(
                                sq2[:, sl], ub2[:, sl], Act.Square
                            )

                for li in range(N_LAYERS):
                    st[("xnb", li)] = t_xnb()
                    S1(li)
                    AG(li)
                    S2(li)
                    S3(li)
                    if li + 2 < N_LAYERS:
                        wts[li + 2] = load_layer_weights(s, li + 2)
                    st.pop(li, None)
                    st.pop(("xnb", li), None)
                    wts.pop(li, None)

            # ---------------- mix + sgate boundary ----------------
            def boundary(k, old_tile):
                mixed = f32a_tile()  # integ dead
                cs = lay.tile([128, DT * TO], f32, tag="cs", name="cs")
                tot = lay.tile([128, DT], f32, tag="tot", name="tot")
                for dtt in range(DT):
                    nc.vector.tensor_tensor_scan(
                        out=cs[:, dtt * TO : (dtt + 1) * TO],
                        data0=xA[:, dtt * TO : (dtt + 1) * TO],
                        data1=xA[:, dtt * TO : (dtt + 1) * TO],
                        initial=0.0,
                        op0=Alu.add,
                        op1=Alu.bypass,
                    )
                    nc.vector.tensor_copy(
                        tot[:, dtt : dtt + 1], cs[:, dtt * TO + TO - 1 : dtt * TO + TO]
                    )
                nc.sync.dma_start(bs_in[:], tot[:])
                nc.gpsimd.collective_compute(
                    "AllGather",
                    mybir.AluOpType.bypass,
                    replica_groups=RG,
                    ins=[bs_in[:]],
                    outs=[bs_out[:]],
                )
                blks = lay.tile([128, 4 * DT], f32, tag="blks", name="blks")
                nc.sync.dma_start(
                    blks[:].rearrange("p (r k) -> p r k", r=4),
                    bs_out[:].rearrange("(r p) k -> p r k", p=128),
                )
                pref = lay.tile([128, DT], f32, tag="pref", name="pref")
                nc.vector.tensor_scalar(
                    pref[:], blks[:, 0:DT], cst[:, 24:25], None, Alu.mult
                )
                for r in range(1, 4):
                    nc.vector.scalar_tensor_tensor(
                        out=pref[:],
                        in0=blks[:, r * DT : (r + 1) * DT],
                        scalar=cst[:, 24 + r : 25 + r],
                        in1=pref[:],
                        op0=Alu.mult,
                        op1=Alu.add,
                    )
                # mixed = xA + (cs + pref) * rc
                for dtt in range(DT):
                    nc.vector.scalar_tensor_tensor(
                        out=cs[:, dtt * TO : (dtt + 1) * TO],
                        in0=cs[:, dtt * TO : (dtt + 1) * TO],
                        scalar=pref[:, dtt : dtt + 1],
                        in1=rc_bc[:],
                        op0=Alu.add,
                        op1=Alu.mult,
                    )
                    nc.gpsimd.tensor_tensor(
                        out=mixed[:, dtt * TO : (dtt + 1) * TO],
                        in0=xA[:, dtt * TO : (dtt + 1) * TO],
                        in1=cs[:, dtt * TO : (dtt + 1) * TO],
                        op=Alu.add,
                    )

                dw_sb = wgt.tile([128, DT * RANK], bf, tag="dw", name="dw")
                for kt in range(DT):
                    nc.sync.dma_start(
                        dw_sb[:, kt * RANK : (kt + 1) * RANK],
                        d_dwT[k, kt * 128 : (kt + 1) * 128, :],
                    )
                uw_sb = wgt.tile([128, D], bf, tag="uw", name="uw")
                nc.sync.dma_start(uw_sb[0:RANK, :], d_uwT[k])

                hsb = lay.tile([128, TO], bf, tag="hsb", name="hsb")
                tgf = t_hb()
                ubx = t_zb()
                nc.scalar.activation(ubx[:], mixed[:], Act.Copy)
                sq = t_sq()
                nc.scalar.activation(sq[:], ubx[:], Act.Square)
                pn = py_tile()
                for kt in range(DT):
                    nc.tensor.matmul(
                        pn[0:1, :],
                        lhsT=ones_bf[:],
                        rhs=sq[:, kt * TO : (kt + 1) * TO],
                        start=(kt == 0),
                        stop=(kt == DT - 1),
                    )
                srow = t_srow()
                nc.scalar.activation(
                    srow[:], pn[0:1, :], Act.Abs_reciprocal_sqrt,
                    bias=epsc[0:1, :], scale=1.0 / D,
                )
                sbc = t_sbc()
                nc.gpsimd.partition_broadcast(sbc[:], srow[0:1, :])
                nb = t_ub()
                for dtt in range(DT):
                    nc.vector.tensor_tensor(
                        out=nb[:, dtt * TO : (dtt + 1) * TO],
                        in0=ubx[:, dtt * TO : (dtt + 1) * TO],
                        in1=sbc[:],
                        op=Alu.mult,
                    )
                psh = pg_tile()
                for kt in range(DT):
                    nc.tensor.matmul(
                        psh[0:RANK, 0:TO],
                        lhsT=dw_sb[:, kt * RANK : (kt + 1) * RANK],
                        rhs=nb[:, kt * TO : (kt + 1) * TO],
                        start=(kt == 0),
                        stop=(kt == DT - 1),
                    )
                nc.scalar.activation(
                    hsb[0:RANK, :],
                    psh[0:RANK, 0:TO],
                    Act.Silu,
                    bias=sgc[0:RANK, k : k + 1],
                )
                for md in range(DT):
                    py = py_tile()
                    nc.tensor.matmul(
                        py[:],
                        lhsT=uw_sb[0:RANK, md * 128 : (md + 1) * 128],
                        rhs=hsb[0:RANK, :],
                        start=True,
                        stop=True,
                    )
                    nc.scalar.activation(
                        tgf[:, md * TO : (md + 1) * TO],
                        py[:],
                        Act.Tanh,
                        scale=0.5,
                        bias=sgc[:, 4 + k * 4 + md : 5 + k * 4 + md],
                    )
                # blend: xA = old + (0.5 + 0.5*t) * (mixed - old)
                dfs = lay.tile([128, DT * TO], bf, tag="dfs", name="dfs", bufs=1)
                nc.vector.tensor_tensor(
                    out=dfs[:], in0=mixed[:], in1=old_tile[:], op=Alu.subtract
                )
                nc.vector.tensor_scalar(tgf[:], tgf[:], 0.5, 0.5, Alu.mult, Alu.add)
                nc.gpsimd.tensor_tensor(out=dfs[:], in0=tgf[:], in1=dfs[:], op=Alu.mult)
                nc.vector.tensor_tensor(
                    out=xA[:], in0=old_tile[:], in1=dfs[:], op=Alu.add
                )

            # ---------------- run the model ----------------
            gated2 = None
            for si in range(N_STACKS):
                run_stack([0, 1, 2][si])
                if si == 0:
                    boundary(0, initial)
                elif si == 1:
                    boundary(1, initial)
                    gated2 = keep_tile()  # initial dead
                    nc.vector.tensor_scalar(gated2[:], xA[:], 1.0, None, Alu.mult)
                elif si == 2:
                    boundary(2, gated2)

            if DEBUG_TRUNK_OUT:
                nc.sync.dma_start(d_trunk[:], xA[:])

            # ---------------- final rmsnorm + AG + tied head ----------------
            if not SKIP_HEAD:
                hd = ctx.enter_context(tc.tile_pool(name="hd", bufs=1))
                ob_own = hd.tile([128, DT * TO], bf, tag="obo", name="obo")
                sq = t_sq()
                nc.scalar.activation(sq[:], xA[:], Act.Square)
                pn = py_tile()
                for kt in range(DT):
                    nc.tensor.matmul(
                        pn[0:1, :],
                        lhsT=ones_bf[:],
                        rhs=sq[:, kt * TO : (kt + 1) * TO],
                        start=(kt == 0),
                        stop=(kt == DT - 1),
                    )
                srow = t_srow()
                nc.scalar.activation(
                    srow[:], pn[0:1, :], Act.Abs_reciprocal_sqrt,
                    bias=epsc[0:1, :], scale=1.0 / D,
                )
                sbc = t_sbc()
                nc.gpsimd.partition_broadcast(sbc[:], srow[0:1, :])
                for dtt in range(DT):
                    nc.vector.tensor_tensor(
                        out=ob_own[:, dtt * TO : (dtt + 1) * TO],
                        in0=xA[:, dtt * TO : (dtt + 1) * TO],
                        in1=sbc[:],
                        op=Alu.mult,
                    )
                agi, ago = agiT[256], agoT[256]
                nc.sync.dma_start(agi[:], ob_own[:])
                nc.gpsimd.collective_compute(
                    "AllGather",
                    mybir.AluOpType.bypass,
                    replica_groups=RG,
                    ins=[agi[:]],
                    outs=[ago[0 : 4 * 128, :]],
                )
                ob = hd.tile([128, DT * T], bf, tag="obf", name="obf")
                ob3 = ob[:].rearrange("p (k t) -> p k t", k=DT)
                for r in range(4):
                    nc.sync.dma_start(
                        ob3[:, :, r * TO : (r + 1) * TO],
                        ago[r * 128 : (r + 1) * 128, :].rearrange(
                            "p (k t) -> p k t", k=DT
                        ),
                    )
                for nv in range(NV):
                    nw = min(512, VS - nv * 512)
                    rhsb = hd.tile([128, DT * 512], bf, tag="rhsb", name="rhsb", bufs=3)
                    nc.sync.dma_start(rhsb[:], d_embT[nv])
                    pcb = pc_tile()
                    for mt in range(8):
                        r = mt % 4
                        if r == 0:
                            psl = pg_tile()
                        elif r == 1:
                            psl = pu_tile()
                        else:
                            psl = pcb[:, 0:512] if r == 2 else pcb[:, 512:1024]
                        for kt in range(DT):
                            nc.tensor.matmul(
                                psl[:, :nw],
                                lhsT=ob[:, kt * T + mt * 128 : kt * T + (mt + 1) * 128],
                                rhs=rhsb[:, kt * 512 : kt * 512 + nw],
                                start=(kt == 0),
                                stop=(kt == DT - 1),
                            )
                        lsb = hd.tile([128, 512], f32, tag="lsb", name="lsb", bufs=4)
                        if mt % 2 == 0:
                            nc.scalar.activation(lsb[:, :nw], psl[:, :nw], Act.Copy)
                        else:
                            nc.vector.tensor_copy(lsb[:, :nw], psl[:, :nw])
                        outq = nc.gpsimd if mt % 2 == 0 else nc.sync
                        outq.dma_start(
                            d_out[mt * 128 : (mt + 1) * 128, nv * 512 : nv * 512 + nw],
                            lsb[:, :nw],
                        )

    nc.finalize()
    _prog_cache[key] = nc
    return nc


def prep_inputs(inputs):
    """Host-side: full model inputs -> list of 8 per-core in_maps."""
    idx = np.asarray(inputs["idx"])
    emb = _f32(inputs["emb"])
    pos = _f32(inputs["pos"])[0, :T]  # [T, D]
    we = _f32(inputs["emb_norm_w"])
    ts = _f32(inputs["token_shift"])
    mgw = _f32(inputs["mem_gate_w"])
    mgb = _f32(inputs["mem_gate_b"])
    memp = _f32(inputs["memory_p"])
    fnw = _f32(inputs["final_norm_w"])

    mgwT = np.ascontiguousarray(mgw.T)

    stack_in = {}
    for s in ("up", "dn"):
        nw = _f32(inputs[f"{s}_norm_w"])  # [NL, D]
        cw = _f32(inputs[f"{s}_conv_w"])[:, :, 0, :]  # [NL, D, K]
        cb = _f32(inputs[f"{s}_conv_b"])  # [NL, D]
        w1 = _f32(inputs[f"{s}_w1"])
        w2 = _f32(inputs[f"{s}_w2"])
        w3 = _f32(inputs[f"{s}_w3"])
        kp = _f32(inputs[f"{s}_kp"])
        ki = _f32(inputs[f"{s}_ki"])
        gn = _f32(inputs[f"{s}_gnorm"])
        cwb = np.zeros((NL, 128, DT), np.float32)
        cwt = np.zeros((NL, 128, KK), np.float32)
        cdiag = np.zeros((NL, 128, DT * KK * 128), np.float32)
        gdiag = np.zeros((NL, 128, DT * 128), np.float32)
        for li in range(NL):
            # conv path: gnorm cancels inside the double rmsnorm; fold norm_w
            # only. gnorm survives only in the residual base (gdiag).
            gfold = gn[li - 1] if li > 0 else np.ones(D, np.float32)
            cwf = cw[li] * nw[li][:, None]  # [D, K]
            taps = cwf[:, ::-1]  # tap m multiplies shift m*d
            cwb[li] = _cols(cb[li])
            cwt[li] = taps[3 * 128 : 4 * 128, :]
            for dtt in range(DT):
                for m in range(KK):
                    cdiag[li, :, (dtt * KK + m) * 128 : (dtt * KK + m + 1) * 128] = np.diag(
                        taps[dtt * 128 : (dtt + 1) * 128, m]
                    )
                gdiag[li, :, dtt * 128 : (dtt + 1) * 128] = np.diag(
                    gfold[dtt * 128 : (dtt + 1) * 128]
                )
        pid = np.zeros((128, (NL - 1) * 12), np.float32)
        for li in range(1, NL):
            pb = (li - 1) * 12
            pid[:, pb : pb + 4] = _cols(kp[li - 1] + ki[li - 1] / li)
            pid[:, pb + 4 : pb + 8] = _cols(ki[li - 1] / li)
        stack_in[f"{s}_cwb"] = np.ascontiguousarray(cwb)
        stack_in[f"{s}_cwt"] = np.ascontiguousarray(cwt)
        stack_in[f"{s}_pid"] = pid
        stack_in[f"{s}_cdiag"] = np.ascontiguousarray(cdiag).astype(BF16)
        stack_in[f"{s}_gdiag"] = np.ascontiguousarray(gdiag).astype(BF16)
        w13 = np.concatenate([w1.transpose(0, 2, 1), w3.transpose(0, 2, 1)], axis=2)
        w13p = np.ascontiguousarray(
            w13.reshape(NL, DT, 128, 2 * HID).transpose(0, 2, 1, 3).reshape(NL, 128, DT * 2 * HID)
        )
        stack_in[f"{s}_w13p"] = w13p.astype(BF16)
        w2T = w2.transpose(0, 2, 1)  # [NL, HID, D]
        w2p = np.ascontiguousarray(
            w2T.reshape(NL, HT, 128, D).transpose(0, 2, 1, 3).reshape(NL, 128, HT * D)
        )
        stack_in[f"{s}_w2p"] = w2p.astype(BF16)

    sgn = _f32(inputs["sg_norm"])
    sgdw = _f32(inputs["sg_down_w"])
    sgdb = _f32(inputs["sg_down_b"])
    sguw = _f32(inputs["sg_up_w"])
    sgub = _f32(inputs["sg_up_b"])
    dwT = np.stack(
        [np.ascontiguousarray(sgdw[k].T * sgn[k][:, None]) for k in range(3)]
    ).astype(BF16)
    uwT = np.stack([np.ascontiguousarray(sguw[k].T) for k in range(3)]).astype(BF16)
    sgc = np.zeros((128, 16), np.float32)
    for k in range(3):
        sgc[0:RANK, k] = sgdb[k]
        sgc[:, 4 + k * 4 : 8 + k * 4] = _cols(0.5 * sgub[k])

    embT = np.ascontiguousarray((emb.T * fnw[:, None]))  # [D, V] f32

    common = dict(
        mgwT=mgwT,
        emb_tbl=emb,
        sg_dwT=dwT,
        sg_uwT=uwT,
        sg_cols=sgc,
        **stack_in,
    )

    in_maps = []
    for c in range(NCORES):
        b = c // 4
        q = c % 4
        beta = q * TO
        m = dict(common)
        # idx: 3 chunks of 128 tokens: [beta-128, beta+256); q=0 pads with idx[0]
        tok = np.arange(beta - 128, beta + TO)
        tokc = np.clip(tok, 0, T - 1)
        m["idx_rs"] = np.ascontiguousarray(
            idx[b][tokc].astype(np.int32).reshape(3, 128).T
        )
        m["pos_rs"] = np.ascontiguousarray(
            pos[tokc].reshape(3, 128, D).transpose(1, 0, 2).reshape(128, 3 * D)
        )
        cst = np.zeros((128, 64), np.float32)
        # col0 coefs: q=0 -> shifted[0]=x[0]: cself0=we, cprev0=0
        if q == 0:
            cst[:, 0:4] = _cols(we)
            cst[:, 12:16] = 0.0
        else:
            cst[:, 0:4] = _cols((1.0 - ts) * we)
            cst[:, 12:16] = _cols(ts * we)
        cst[:, 4:8] = _cols(ts * we)
        cst[:, 8:12] = _cols((1.0 - ts) * we)
        cst[:, 16:20] = _cols(0.5 * mgb)
        cst[:, 20:24] = _cols(memp[b])
        for r in range(4):
            cst[:, 24 + r] = 1.0 if r < q else 0.0
        m["cst"] = cst
        rc = (MIX_W / (np.arange(beta + 1, beta + TO + 1, dtype=np.float32)))[None, :]
        m["rc_bc"] = np.ascontiguousarray(np.broadcast_to(rc, (128, TO)))
        # halo row-offset tables: rows into ago [640, ...]; block 4 is zeros
        offs = np.zeros((128, 2), np.int32)
        p = np.arange(128, dtype=np.int32)
        offs[:, 0] = (q - 1) * 128 + p if q >= 1 else 4 * 128 + p
        offs[:, 1] = (q - 2) * 128 + p if q >= 2 else 4 * 128 + p
        m["halo_offs"] = offs
        # head vocab shard
        esh = embT[:, q * VS : (q + 1) * VS]  # [D, VS]
        eshpad = np.zeros((D, NV * 512), np.float32)
        eshpad[:, :VS] = esh
        eshp = np.ascontiguousarray(
            eshpad.reshape(DT, 128, NV, 512).transpose(2, 1, 0, 3).reshape(NV, 128, DT * 512)
        )
        m["embT_sh"] = eshp.astype(BF16)
        in_maps.append(m)
    return in_maps


LAST_RESULTS = None


def kernel(**inputs):
    global LAST_RESULTS
    from concourse.bass_utils import run_bass_kernel_spmd

    nc = build_program()
    in_maps = prep_inputs(inputs)
    trace = bool(int(os.environ.get("KB_TRACE", "0")))
    res = run_bass_kernel_spmd(nc, in_maps, core_ids=list(range(NCORES)), trace=trace)
    LAST_RESULTS = res
    out = np.zeros((B, T, VOCAB), np.float32)
    for c in range(NCORES):
        b = c // 4
        q = c % 4
        out[b, :, q * VS : (q + 1) * VS] = res.results[c]["logits_sh"]
    return out


# revision 21
# speedup vs baseline: 1.0126x; 1.0075x over previous
"""Trainium2 Bass kernel for nn_MemPIDModel (dense_cnn) — sequence-parallel.

Strategy (8 NeuronCores):
  - core c handles sample b = c//4, token quarter q = c%4 (256 tokens each)
  - trunk is sequence-parallel: per layer, each core computes its 256-token
    slice; the causal dilated conv's halo is exchanged via a 4-rank
    AllGather of the normed conv input (bf16), sized to the layer's actual
    receptive field h = min(14*dil, 256) tokens
  - halo readback: indirect DMA with per-core row-offset tables (an
    always-zero block in each AG buffer provides causal zero padding, so
    one SPMD program works for all cores); full-row gathers land strided
    across the 4 d-tiles in one instruction
  - conv runs fully on PE as 15 diag-matmul accumulations per d-tile at
    256-col streams (matmul issue cadence bound -> widest possible)
  - mix boundaries: local cumsum scan + tiny AllGather of slice totals for
    the cross-core prefix (mask-weighted sum keeps the program uniform)
  - head: AllGather of the normed trunk output, then each core computes its
    vocab shard v = c%4 over all 1024 tokens ([1024,512]@[512,8000])
"""

import os
import sys
import numpy as np

sys.path.insert(0, "/opt/trn_rl_repo")

import ml_dtypes

B = 2
T = 1024
D = 512
HID = 1024
KK = 15
VOCAB = 32000
RANK = 64
NL = 6
MIX_W = 0.1
UP_DIL = [1, 2, 4, 8, 16, 32]
DN_DIL = UP_DIL[::-1]
EPS = 1e-6
NCORES = 8
VSHARDS = 4
VS = VOCAB // VSHARDS  # 8000
DT = D // 128  # 4 D-tiles
HT = HID // 128  # 8 H-tiles
TO = 256  # own tokens per core
POFF = 512  # own region offset inside xnb (per d-tile)
CONVW = POFF + TO  # 768 padded conv input width per d-tile
NV = (VS + 511) // 512  # 16 head column chunks (last is 320 wide)
HS = sorted({min(14 * d, 256) for d in UP_DIL})  # AG payload widths

BF16 = ml_dtypes.bfloat16

# debug knobs (affect program shape; kernel cache key includes them)
N_LAYERS = int(os.environ.get("KB_LAYERS", str(NL)))
N_STACKS = int(os.environ.get("KB_STACKS", "3"))
SKIP_HEAD = bool(int(os.environ.get("KB_SKIP_HEAD", "0")))
DEBUG_TRUNK_OUT = bool(int(os.environ.get("KB_TRUNK_OUT", "0")))

_prog_cache = {}


def _f32(x):
    return np.ascontiguousarray(np.asarray(x), dtype=np.float32)


def _cols(v):
    """[D] vector -> [128, DT] A-layout per-partition columns."""
    return np.ascontiguousarray(_f32(v).reshape(DT, 128).T)


def _stack_dils(stack_idx):
    return UP_DIL if stack_idx in (0, 2) else DN_DIL


def build_program():
    key = (N_LAYERS, N_STACKS, SKIP_HEAD, DEBUG_TRUNK_OUT)
    if key in _prog_cache:
        return _prog_cache[key]

    import concourse.bass as bass
    import concourse.mybir as mybir
    import concourse.tile as tile
    from concourse import bacc
    from concourse.masks import make_identity

    dt = mybir.dt
    Alu = mybir.AluOpType
    Act = mybir.ActivationFunctionType

    nc = bacc.Bacc(None, target_bir_lowering=False, debug=False)

    RG = [[0, 1, 2, 3], [4, 5, 6, 7]]

    # ---------------- DRAM I/O ----------------
    d_idx = nc.dram_tensor("idx_rs", [128, 3], dt.int32, kind="ExternalInput")
    d_emb = nc.dram_tensor("emb_tbl", [VOCAB, D], dt.float32, kind="ExternalInput")
    d_pos = nc.dram_tensor("pos_rs", [128, 3 * D], dt.float32, kind="ExternalInput")
    d_cst = nc.dram_tensor("cst", [128, 64], dt.float32, kind="ExternalInput")
    d_rc = nc.dram_tensor("rc_bc", [128, TO], dt.float32, kind="ExternalInput")
    d_mgw = nc.dram_tensor("mgwT", [D, D], dt.float32, kind="ExternalInput")
    d_offs = nc.dram_tensor("halo_offs", [128, 2], dt.int32, kind="ExternalInput")

    d_cwb = {}
    d_cwt = {}
    d_w13 = {}
    d_w2 = {}
    d_pid = {}
    d_cdiag = {}
    d_gdiag = {}
    for s in ("up", "dn"):
        d_cwb[s] = nc.dram_tensor(
            f"{s}_cwb", [NL, 128, DT], dt.float32, kind="ExternalInput"
        )
        d_cwt[s] = nc.dram_tensor(
            f"{s}_cwt", [NL, 128, KK], dt.float32, kind="ExternalInput"
        )
        d_w13[s] = nc.dram_tensor(
            f"{s}_w13p", [NL, 128, DT * 2 * HID], dt.bfloat16, kind="ExternalInput"
        )
        d_w2[s] = nc.dram_tensor(
            f"{s}_w2p", [NL, 128, HT * D], dt.bfloat16, kind="ExternalInput"
        )
        d_pid[s] = nc.dram_tensor(
            f"{s}_pid", [128, (NL - 1) * 12], dt.float32, kind="ExternalInput"
        )
        d_cdiag[s] = nc.dram_tensor(
            f"{s}_cdiag", [NL, 128, DT * KK * 128], dt.bfloat16, kind="ExternalInput"
        )
        d_gdiag[s] = nc.dram_tensor(
            f"{s}_gdiag", [NL, 128, DT * 128], dt.bfloat16, kind="ExternalInput"
        )
    d_dwT = nc.dram_tensor("sg_dwT", [3, D, RANK], dt.bfloat16, kind="ExternalInput")
    d_uwT = nc.dram_tensor("sg_uwT", [3, RANK, D], dt.bfloat16, kind="ExternalInput")
    d_sgc = nc.dram_tensor("sg_cols", [128, 16], dt.float32, kind="ExternalInput")
    d_embT = nc.dram_tensor("embT_sh", [NV, 128, DT * 512], dt.bfloat16, kind="ExternalInput")

    d_out = nc.dram_tensor("logits_sh", [T, VS], dt.float32, kind="ExternalOutput")
    if DEBUG_TRUNK_OUT:
        d_trunk = nc.dram_tensor("trunk_out", [128, DT * TO], dt.float32, kind="ExternalOutput")

    f32 = dt.float32
    bf = dt.bfloat16

    with tile.TileContext(nc) as tc:
        import contextlib

        ctx = contextlib.ExitStack()
        with ctx:
            const = ctx.enter_context(tc.tile_pool(name="const", bufs=1))
            master = ctx.enter_context(tc.tile_pool(name="master", bufs=1))
            lay = ctx.enter_context(tc.tile_pool(name="lay", bufs=1))
            wgt = ctx.enter_context(tc.tile_pool(name="wgt", bufs=2))
            psum = ctx.enter_context(tc.tile_pool(name="psum", bufs=1, space="PSUM"))
            dram = ctx.enter_context(tc.tile_pool(name="dram", bufs=1, space="DRAM"))

            # ---------------- constants ----------------
            epsc = const.tile([128, 1], f32, tag="epsc")
            nc.vector.memset(epsc[:], EPS)
            ones_bf = const.tile([128, 1], bf, tag="ones")
            nc.vector.memset(ones_bf[:], 1.0)
            ones_row = const.tile([1, 128], bf, tag="onesr")
            nc.vector.memset(ones_row[:], 1.0)
            ident = const.tile([128, 128], f32, tag="ident")
            make_identity(nc, ident[:])
            cst = const.tile([128, 64], f32, tag="cst")
            nc.sync.dma_start(cst[:], d_cst[:])
            rc_bc = const.tile([128, TO], f32, tag="rc")
            nc.sync.dma_start(rc_bc[:], d_rc[:])
            sgc = const.tile([128, 16], f32, tag="sgc")
            nc.sync.dma_start(sgc[:], d_sgc[:])
            offs = const.tile([128, 2], dt.int32, tag="offs")
            nc.sync.dma_start(offs[:], d_offs[:])
            pidc = {}
            for s in ("up", "dn"):
                pidc[s] = const.tile(
                    [128, (NL - 1) * 12], f32, tag=f"pid_{s}", name=f"pid_{s}"
                )
                nc.sync.dma_start(pidc[s][:], d_pid[s][:])

            # AllGather buffers per payload width h: out rows [0:512] written
            # by AG, rows [512:640] stay zero (causal pad source)
            zero_sb = const.tile([128, DT * TO], bf, tag="zsb")
            nc.gpsimd.memset(zero_sb[:], 0.0)
            agiT = {}
            agoT = {}
            for h in HS:
                agiT[h] = dram.tile([128, DT * h], bf, tag=f"agi{h}", name=f"agi{h}")
                agoT[h] = dram.tile([5 * 128, DT * h], bf, tag=f"ago{h}", name=f"ago{h}")
                nc.sync.dma_start(agoT[h][4 * 128 : 5 * 128, :], zero_sb[:, 0 : DT * h])
            bs_in = dram.tile([128, DT], f32, tag="bs_in", bufs=1)
            bs_out = dram.tile([4 * 128, DT], f32, tag="bs_out", bufs=1)

            # persistent activations (A-layout, free index = dt*TO + t)
            xA = master.tile([128, DT * TO], f32, tag="xA")

            def keep_tile():  # initial, then gated2 (sequential lifetimes)
                return master.tile([128, DT * TO], f32, tag="keep", name="keep")

            def f32a_tile():  # integ during stacks / mixed during boundaries
                return lay.tile([128, DT * TO], f32, tag="f32a", name="f32a")

            def t_zb():
                return lay.tile([128, DT * TO], bf, tag="zb", name="zb", bufs=2)

            def t_ub():
                return lay.tile([128, DT * TO], bf, tag="ub", name="ub", bufs=2)

            def t_sq():
                return lay.tile([128, DT * TO], bf, tag="sq", name="sq", bufs=2)

            def t_hb():
                return lay.tile([128, DT * TO], bf, tag="hb", name="hb", bufs=2)

            def t_sbc():
                return lay.tile([128, TO], bf, tag="sbc", name="sbc", bufs=2)

            def t_srow():
                return lay.tile([1, TO], bf, tag="srow", name="srow", bufs=2)

            def t_pch():
                return lay.tile([128, HT * TO], bf, tag="pch", name="pch", bufs=2)

            def t_gst():
                return lay.tile([128, 2 * TO], bf, tag="gst", name="gst", bufs=2)

            def t_xnb():
                return lay.tile([128, DT * CONVW], bf, tag="xnb", name="xnb", bufs=2)

            # PSUM: pc [128,1024]=2 banks, pg/pu/py [*,<=512] bufs=2
            def pc_tile():
                return psum.tile([128, DT * TO], f32, tag="pc", bufs=1, name="pc")

            def pg_tile():
                return psum.tile([128, 2 * TO], f32, tag="pg", bufs=2, name="pg")

            def pu_tile():
                return psum.tile([128, 2 * TO], f32, tag="pu", bufs=2, name="pu")

            def py_tile():
                return psum.tile([128, TO], f32, tag="py", bufs=2, name="py")

            # ---------------- P0: gather + embnorm + shift + mem ----------------
            with tc.tile_pool(name="p0", bufs=1) as p0:
                idx_sb = p0.tile([128, 3], dt.int32, tag="idx")
                nc.sync.dma_start(idx_sb[:], d_idx[:])
                gth = p0.tile([128, 3 * D], f32, tag="gth")
                for c in range(3):
                    nc.gpsimd.indirect_dma_start(
                        out=gth[:, c * D : (c + 1) * D],
                        out_offset=None,
                        in_=d_emb[:],
                        in_offset=bass.IndirectOffsetOnAxis(ap=idx_sb[:, c : c + 1], axis=0),
                    )
                pos_sb = p0.tile([128, 3 * D], f32, tag="pos")
                nc.sync.dma_start(pos_sb[:], d_pos[:])
                nc.vector.tensor_tensor(
                    out=gth[:], in0=gth[:], in1=pos_sb[:], op=Alu.add
                )
                ss = p0.tile([128, 3], f32, tag="ss")
                sqt = p0.tile([128, D], f32, tag="sqt")
                for c in range(3):
                    nc.scalar.activation(
                        sqt[:],
                        gth[:, c * D : (c + 1) * D],
                        Act.Square,
                        accum_out=ss[:, c : c + 1],
                    )
                nc.scalar.activation(ss[:], ss[:], Act.Ln, bias=epsc[:], scale=1.0 / D)
                nc.scalar.activation(ss[:], ss[:], Act.Exp, scale=-0.5)
                for c in range(3):
                    nc.vector.tensor_scalar(
                        gth[:, c * D : (c + 1) * D],
                        gth[:, c * D : (c + 1) * D],
                        ss[:, c : c + 1],
                        None,
                        Alu.mult,
                    )
                # transpose token-major -> A-layout x_n [128, DT*384]
                x_n = p0.tile([128, DT * 384], f32, tag="xn")
                for c in range(3):
                    pst = pg_tile()
                    for dtt in range(2):
                        nc.tensor.transpose(
                            out=pst[:, dtt * 128 : (dtt + 1) * 128],
                            in_=gth[:, c * D + dtt * 128 : c * D + (dtt + 1) * 128],
                            identity=ident[:],
                        )
                    pst2 = pu_tile()
                    for dtt in range(2):
                        nc.tensor.transpose(
                            out=pst2[:, dtt * 128 : (dtt + 1) * 128],
                            in_=gth[:, c * D + (2 + dtt) * 128 : c * D + (3 + dtt) * 128],
                            identity=ident[:],
                        )
                    for dtt in range(2):
                        nc.vector.tensor_copy(
                            x_n[:, dtt * 384 + c * 128 : dtt * 384 + (c + 1) * 128],
                            pst[:, dtt * 128 : (dtt + 1) * 128],
                        )
                        nc.vector.tensor_copy(
                            x_n[:, (2 + dtt) * 384 + c * 128 : (2 + dtt) * 384 + (c + 1) * 128],
                            pst2[:, dtt * 128 : (dtt + 1) * 128],
                        )
                # mem gate: mem = sigmoid(memp @ mgw.T + mgb)
                ps_mem = py_tile()
                mgw_sb = p0.tile([128, DT * D], f32, tag="mgw", name="mgw_sb")
                nc.scalar.dma_start(
                    mgw_sb[:].rearrange("p (k d) -> p k d", k=DT),
                    d_mgw[:].rearrange("(k p) d -> p k d", p=128),
                )
                for kt in range(DT):
                    for m in range(DT):
                        nc.tensor.matmul(
                            ps_mem[:, m : m + 1],
                            lhsT=mgw_sb[:, kt * D + m * 128 : kt * D + (m + 1) * 128],
                            rhs=cst[:, 20 + kt : 21 + kt],
                            start=(kt == 0),
                            stop=(kt == DT - 1),
                        )
                tmem = p0.tile([128, 4], f32, tag="tmem")
                for m in range(DT):
                    nc.scalar.activation(
                        tmem[:, m : m + 1],
                        ps_mem[:, m : m + 1],
                        Act.Tanh,
                        scale=0.5,
                        bias=cst[:, 16 + m : 17 + m],
                    )
                nc.vector.tensor_scalar(tmem[:], tmem[:], 0.5, 0.5, Alu.mult, Alu.add)
                # token shift + mem (own tokens live at x_n cols 128..384)
                tsh = p0.tile([128, TO], f32, tag="tsh")
                for dtt in range(DT):
                    ox = dtt * 384 + 128  # own region in x_n
                    oa = dtt * TO  # xA
                    nc.vector.tensor_scalar(
                        tsh[:, 0:1], x_n[:, ox - 1 : ox], cst[:, 12 + dtt : 13 + dtt],
                        None, Alu.mult,
                    )
                    nc.vector.scalar_tensor_tensor(
                        out=xA[:, oa : oa + 1],
                        in0=x_n[:, ox : ox + 1],
                        scalar=cst[:, dtt : dtt + 1],
                        in1=tsh[:, 0:1],
                        op0=Alu.mult,
                        op1=Alu.add,
                    )
                    nc.vector.tensor_scalar(
                        tsh[:, 1:TO],
                        x_n[:, ox + 1 : ox + TO],
                        cst[:, 8 + dtt : 9 + dtt],
                        None,
                        Alu.mult,
                    )
                    nc.vector.scalar_tensor_tensor(
                        out=xA[:, oa + 1 : oa + TO],
                        in0=x_n[:, ox : ox + TO - 1],
                        scalar=cst[:, 4 + dtt : 5 + dtt],
                        in1=tsh[:, 1:TO],
                        op0=Alu.mult,
                        op1=Alu.add,
                    )
                    nc.vector.tensor_scalar(
                        xA[:, oa : oa + TO], xA[:, oa : oa + TO],
                        tmem[:, dtt : dtt + 1], None, Alu.add,
                    )
            initial = keep_tile()
            nc.vector.tensor_scalar(initial[:], xA[:], 1.0, None, Alu.mult)

            # ---------------- conv block stack ----------------
            def load_layer_weights(s, li, q2=None):
                w = {}
                if q2 is None:
                    q2 = nc.scalar
                w["cwb"] = wgt.tile([128, DT], f32, tag="cwb", name="cwb")
                nc.sync.dma_start(w["cwb"][:], d_cwb[s][li])
                w["cwt"] = wgt.tile([128, KK], f32, tag="cwt", name="cwt")
                nc.sync.dma_start(w["cwt"][:], d_cwt[s][li])
                w["w13"] = wgt.tile([128, DT * 2 * HID], bf, tag="w13", name="w13")
                for qq in range(2):
                    qs = qq * 4 * HID
                    nc.sync.dma_start(
                        w["w13"][:, qs : qs + 4 * HID], d_w13[s][li, :, qs : qs + 4 * HID]
                    )
                w["w2"] = wgt.tile([128, HT * D], bf, tag="w2", name="w2")
                q2.dma_start(w["w2"][:], d_w2[s][li])
                w["cdiag"] = wgt.tile([128, DT * KK * 128], bf, tag="cdiag", name="cdiag")
                q2.dma_start(w["cdiag"][:], d_cdiag[s][li])
                w["gdiag"] = wgt.tile([128, DT * 128], bf, tag="gdiag", name="gdiag")
                q2.dma_start(w["gdiag"][:], d_gdiag[s][li])
                return w

            def run_stack(stack_idx):
                s = "up" if stack_idx in (0, 2) else "dn"
                dils = _stack_dils(stack_idx)
                integ = f32a_tile()
                nc.vector.tensor_scalar(integ[:], xA[:], 1.0, None, Alu.mult)

                wts = {}
                for li in range(min(2, N_LAYERS)):
                    wts[li] = load_layer_weights(s, li, q2=nc.sync)

                st = {}
                pend_integ = []

                def flush_integ():
                    for sl in pend_integ:
                        nc.gpsimd.tensor_tensor(
                            out=integ[:, sl], in0=integ[:, sl],
                            in1=xA[:, sl], op=Alu.add,
                        )
                    pend_integ.clear()

                def S1(li):
                    # rmsnorm scale -> xnb own region (ub/sq made by S3 tail
                    # of the previous layer for li>0)
                    cs = st.setdefault(li, {})
                    if li == 0:
                        ub = cs["ub"] = t_ub()
                        nc.scalar.activation(ub[:], xA[:], Act.Copy)
                        sq = cs["sq"] = t_sq()
                        nc.scalar.activation(sq[:], ub[:], Act.Square)
                    ub = cs["ub"]
                    sq = cs["sq"]
                    pn = py_tile()
                    for kt in range(DT):
                        nc.tensor.matmul(
                            pn[0:1, :],
                            lhsT=ones_bf[:],
                            rhs=sq[:, kt * TO : (kt + 1) * TO],
                            start=(kt == 0),
                            stop=(kt == DT - 1),
                        )
                    srow = t_srow()
                    nc.scalar.activation(
                        srow[:], pn[0:1, :], Act.Abs_reciprocal_sqrt,
                        bias=epsc[0:1, :], scale=1.0 / D,
                    )
                    # broadcast via 1-row PE matmul (keeps gpsimd queue clear)
                    sbc = py_tile()
                    nc.tensor.matmul(
                        sbc[:], lhsT=ones_row[0:1, :], rhs=srow[0:1, :],
                        start=True, stop=True,
                    )
                    xnb = st[("xnb", li)]
                    for dtt in range(DT):
                        nc.vector.tensor_tensor(
                            out=xnb[:, dtt * CONVW + POFF : dtt * CONVW + POFF + TO],
                            in0=ub[:, dtt * TO : (dtt + 1) * TO],
                            in1=sbc[:],
                            op=Alu.mult,
                        )

                def AG(li):
                    # exchange last h own tokens; read back h-token halo
                    d = dils[li]
                    h = min(14 * d, TO)
                    xnb = st[("xnb", li)]
                    agi, ago = agiT[h], agoT[h]
                    xnb3 = xnb[:].rearrange("p (d w) -> p d w", d=DT)
                    nc.sync.dma_start(agi[:], xnb3[:, :, POFF + TO - h : POFF + TO])
                    nc.gpsimd.collective_compute(
                        "AllGather",
                        mybir.AluOpType.bypass,
                        replica_groups=RG,
                        ins=[agi[:]],
                        outs=[ago[0 : 4 * 128, :]],
                    )
                    # prev1: its last h tokens -> staging -> xnb [POFF-h, POFF)
                    stg = lay.tile([128, DT * TO], bf, tag="stg", name="stg", bufs=2)
                    nc.gpsimd.indirect_dma_start(
                        out=stg[:, 0 : DT * h],
                        out_offset=None,
                        in_=ago[:],
                        in_offset=bass.IndirectOffsetOnAxis(ap=offs[:, 0:1], axis=0),
                        element_offset=0,
                    )
                    stg3 = stg[:, 0 : DT * h].rearrange("p (d w) -> p d w", d=DT)
                    nc.vector.tensor_copy(xnb3[:, :, POFF - h : POFF], stg3[:, :, :])
                    if 14 * d > TO:
                        # d=32: prev2 tokens [beta-448,beta-256) via its cols [64,256)
                        stg2 = lay.tile([128, DT * TO], bf, tag="stg", name="stg2", bufs=2)
                        nc.gpsimd.indirect_dma_start(
                            out=stg2[:, 0 : DT * h],
                            out_offset=None,
                            in_=ago[:],
                            in_offset=bass.IndirectOffsetOnAxis(ap=offs[:, 1:2], axis=0),
                            element_offset=0,
                        )
                        stg23 = stg2[:, 0 : DT * h].rearrange("p (d w) -> p d w", d=DT)
                        nc.vector.tensor_copy(
                            xnb3[:, :, 64:TO], stg23[:, :, 64:TO]
                        )

                def S2(li):
                    # conv: dtiles 0-2 on PE (diag matmuls, own/halo col-split
                    # for d>=16 so own-col work runs during the AllGather);
                    # dtile 3 on Pool as an STT tap chain
                    d = dils[li]
                    cs = st[li]
                    w = wts[li]
                    xnb = st[("xnb", li)]
                    cacc = lay.tile([128, TO], bf, tag="cacc", name="cacc", bufs=2)
                    ob3 = 3 * CONVW

                    def tap_in(m):
                        stt = ob3 + POFF - m * d
                        return xnb[:, stt : stt + TO]

                    nc.vector.tensor_scalar(
                        cacc[:], tap_in(KK - 1), w["cwt"][:, KK - 1 : KK], None, Alu.mult
                    )
                    for m in range(KK - 2, -1, -1):
                        nc.vector.scalar_tensor_tensor(
                            out=cacc[:],
                            in0=tap_in(m),
                            scalar=w["cwt"][:, m : m + 1],
                            in1=cacc[:],
                            op0=Alu.mult,
                            op1=Alu.add,
                        )
                    psc = pc_tile()
                    for dtt in range(3):
                        ob = dtt * CONVW
                        oc = dtt * TO
                        if d >= 16:
                            # own-col parts first (no halo dependency)
                            for m in range(KK):
                                lo = m * d
                                if lo >= TO:
                                    continue
                                nc.tensor.matmul(
                                    psc[:, oc + lo : oc + TO],
                                    lhsT=w["cdiag"][:, (dtt * KK + m) * 128 : (dtt * KK + m + 1) * 128],
                                    rhs=xnb[:, ob + POFF : ob + POFF + TO - lo],
                                    start=(m == 0),
                                    stop=(m == KK - 1),
                                )
                            for m in range(1, KK):
                                lo = min(m * d, TO)
                                nc.tensor.matmul(
                                    psc[:, oc : oc + lo],
                                    lhsT=w["cdiag"][:, (dtt * KK + m) * 128 : (dtt * KK + m + 1) * 128],
                                    rhs=xnb[:, ob + POFF - m * d : ob + POFF - m * d + lo],
                                    start=False,
                                    stop=(m == KK - 1),
                                )
                        else:
                            for m in range(KK - 1, -1, -1):
                                stt = ob + POFF - m * d
                                nc.tensor.matmul(
                                    psc[:, oc : oc + TO],
                                    lhsT=w["cdiag"][:, (dtt * KK + m) * 128 : (dtt * KK + m + 1) * 128],
                                    rhs=xnb[:, stt : stt + TO],
                                    start=(m == KK - 1),
                                    stop=(m == 0),
                                )
                    hb = cs["hb"] = t_hb()
                    for dtt in range(3):
                        nc.scalar.activation(
                            hb[:, dtt * TO : (dtt + 1) * TO],
                            psc[:, dtt * TO : (dtt + 1) * TO],
                            Act.Silu,
                            bias=w["cwb"][:, dtt : dtt + 1],
                        )
                    nc.scalar.activation(
                        hb[:, 3 * TO : 4 * TO],
                        cacc[:],
                        Act.Silu,
                        bias=w["cwb"][:, 3:4],
                    )

                def S3(li):
                    cs = st[li]
                    w = wts[li]
                    xnb = st[("xnb", li)]
                    hb = cs["hb"]
                    pch = t_pch()
                    for pr in range(4):
                        pg = pg_tile()
                        for j in range(2):
                            kh = pr * 2 + j
                            for kt in range(DT):
                                nc.tensor.matmul(
                                    pg[:, j * TO : (j + 1) * TO],
                                    lhsT=w["w13"][:, kt * 2 * HID + kh * 128 : kt * 2 * HID + (kh + 1) * 128],
                                    rhs=hb[:, kt * TO : (kt + 1) * TO],
                                    start=(kt == 0),
                                    stop=(kt == DT - 1),
                                )
                        gst = t_gst()
                        nc.scalar.activation(gst[:], pg[:], Act.Silu)
                        pu = pu_tile()
                        for j in range(2):
                            kh = pr * 2 + j
                            for kt in range(DT):
                                nc.tensor.matmul(
                                    pu[:, j * TO : (j + 1) * TO],
                                    lhsT=w["w13"][:, kt * 2 * HID + HID + kh * 128 : kt * 2 * HID + HID + (kh + 1) * 128],
                                    rhs=hb[:, kt * TO : (kt + 1) * TO],
                                    start=(kt == 0),
                                    stop=(kt == DT - 1),
                                )
                        nc.vector.tensor_tensor(
                            out=pch[:, pr * 2 * TO : (pr + 1) * 2 * TO],
                            in0=gst[:],
                            in1=pu[:],
                            op=Alu.mult,
                        )
                    for md in range(DT):
                        py = py_tile()
                        if li > 0:
                            base = xnb[:, md * CONVW + POFF : md * CONVW + POFF + TO]
                        else:
                            base = cs["ub"][:, md * TO : (md + 1) * TO]
                        nc.tensor.matmul(
                            py[:],
                            lhsT=w["gdiag"][:, md * 128 : (md + 1) * 128],
                            rhs=base,
                            start=True,
                            stop=False,
                        )
                        for kh in range(HT):
                            nc.tensor.matmul(
                                py[:],
                                lhsT=w["w2"][:, kh * D + md * 128 : kh * D + (md + 1) * 128],
                                rhs=pch[:, kh * TO : (kh + 1) * TO],
                                start=False,
                                stop=(kh == HT - 1),
                            )
                        xs = xA[:, md * TO : (md + 1) * TO]
                        if md % 2 == 0:
                            nc.scalar.activation(xs, py[:], Act.Copy)
                        else:
                            nc.vector.tensor_copy(xs, py[:])
                    # tail: per d-tile integ update + next layer's PID/silu/sq
                    if li < N_LAYERS - 1:
                        pc = pidc[s]
                        pb = li * 12
                        zb = t_zb()
                        nxt = st.setdefault(li + 1, {})
                        ub2 = nxt["ub"] = t_ub()
                        sq2 = nxt["sq"] = t_sq()
                        for dtt in range(DT):
                            sl = slice(dtt * TO, (dtt + 1) * TO)
                            # zb = (kp+ki')*xA + ki'*integ_old
                            nc.vector.tensor_scalar(
                                zb[:, sl], xA[:, sl],
                                pc[:, pb + dtt : pb + 1 + dtt], None, Alu.mult,
                            )
                            nc.vector.scalar_tensor_tensor(
                                out=zb[:, sl],
                                in0=integ[:, sl],
                                scalar=pc[:, pb + 4 + dtt : pb + 5 + dtt],
                                in1=zb[:, sl],
                                op0=Alu.mult,
                                op1=Alu.add,
                            )
                            if li + 2 < N_LAYERS:
                                pend_integ.append(sl)
                            nc.scalar.activation(
                                ub2[:, sl], zb[:, sl], Act.Silu
                            )
                            nc.scalar.activation(
                                sq2[:, sl], ub2[:, sl], Act.Square
                            )

                for li in range(N_LAYERS):
                    st[("xnb", li)] = t_xnb()
                    S1(li)
                    AG(li)
                    flush_integ()
                    S2(li)
                    S3(li)
                    if li + 2 < N_LAYERS:
                        wts[li + 2] = load_layer_weights(s, li + 2)
                    st.pop(li, None)
                    st.pop(("xnb", li), None)
                    wts.pop(li, None)

            # ---------------- mix + sgate boundary ----------------
            def boundary(k, old_tile):
                mixed = f32a_tile()  # integ dead
                cs = lay.tile([128, DT * TO], f32, tag="cs", name="cs")
                tot = lay.tile([128, DT], f32, tag="tot", name="tot")
                for dtt in range(DT):
                    nc.vector.tensor_tensor_scan(
                        out=cs[:, dtt * TO : (dtt + 1) * TO],
                        data0=xA[:, dtt * TO : (dtt + 1) * TO],
                        data1=xA[:, dtt * TO : (dtt + 1) * TO],
                        initial=0.0,
                        op0=Alu.add,
                        op1=Alu.bypass,
                    )
                    nc.vector.tensor_copy(
                        tot[:, dtt : dtt + 1], cs[:, dtt * TO + TO - 1 : dtt * TO + TO]
                    )
                nc.sync.dma_start(bs_in[:], tot[:])
                nc.gpsimd.collective_compute(
                    "AllGather",
                    mybir.AluOpType.bypass,
                    replica_groups=RG,
                    ins=[bs_in[:]],
                    outs=[bs_out[:]],
                )
                blks = lay.tile([128, 4 * DT], f32, tag="blks", name="blks")
                nc.sync.dma_start(
                    blks[:].rearrange("p (r k) -> p r k", r=4),
                    bs_out[:].rearrange("(r p) k -> p r k", p=128),
                )
                pref = lay.tile([128, DT], f32, tag="pref", name="pref")
                nc.vector.tensor_scalar(
                    pref[:], blks[:, 0:DT], cst[:, 24:25], None, Alu.mult
                )
                for r in range(1, 4):
                    nc.vector.scalar_tensor_tensor(
                        out=pref[:],
                        in0=blks[:, r * DT : (r + 1) * DT],
                        scalar=cst[:, 24 + r : 25 + r],
                        in1=pref[:],
                        op0=Alu.mult,
                        op1=Alu.add,
                    )
                # mixed = xA + (cs + pref) * rc
                for dtt in range(DT):
                    nc.vector.scalar_tensor_tensor(
                        out=cs[:, dtt * TO : (dtt + 1) * TO],
                        in0=cs[:, dtt * TO : (dtt + 1) * TO],
                        scalar=pref[:, dtt : dtt + 1],
                        in1=rc_bc[:],
                        op0=Alu.add,
                        op1=Alu.mult,
                    )
                    nc.gpsimd.tensor_tensor(
                        out=mixed[:, dtt * TO : (dtt + 1) * TO],
                        in0=xA[:, dtt * TO : (dtt + 1) * TO],
                        in1=cs[:, dtt * TO : (dtt + 1) * TO],
                        op=Alu.add,
                    )

                dw_sb = wgt.tile([128, DT * RANK], bf, tag="dw", name="dw")
                for kt in range(DT):
                    nc.sync.dma_start(
                        dw_sb[:, kt * RANK : (kt + 1) * RANK],
                        d_dwT[k, kt * 128 : (kt + 1) * 128, :],
                    )
                uw_sb = wgt.tile([128, D], bf, tag="uw", name="uw")
                nc.sync.dma_start(uw_sb[0:RANK, :], d_uwT[k])

                hsb = lay.tile([128, TO], bf, tag="hsb", name="hsb")
                tgf = t_hb()
                ubx = t_zb()
                nc.scalar.activation(ubx[:], mixed[:], Act.Copy)
                sq = t_sq()
                nc.scalar.activation(sq[:], ubx[:], Act.Square)
                pn = py_tile()
                for kt in range(DT):
                    nc.tensor.matmul(
                        pn[0:1, :],
                        lhsT=ones_bf[:],
                        rhs=sq[:, kt * TO : (kt + 1) * TO],
                        start=(kt == 0),
                        stop=(kt == DT - 1),
                    )
                srow = t_srow()
                nc.scalar.activation(
                    srow[:], pn[0:1, :], Act.Abs_reciprocal_sqrt,
                    bias=epsc[0:1, :], scale=1.0 / D,
                )
                sbc = t_sbc()
                nc.gpsimd.partition_broadcast(sbc[:], srow[0:1, :])
                nb = t_ub()
                for dtt in range(DT):
                    nc.vector.tensor_tensor(
                        out=nb[:, dtt * TO : (dtt + 1) * TO],
                        in0=ubx[:, dtt * TO : (dtt + 1) * TO],
                        in1=sbc[:],
                        op=Alu.mult,
                    )
                psh = pg_tile()
                for kt in range(DT):
                    nc.tensor.matmul(
                        psh[0:RANK, 0:TO],
                        lhsT=dw_sb[:, kt * RANK : (kt + 1) * RANK],
                        rhs=nb[:, kt * TO : (kt + 1) * TO],
                        start=(kt == 0),
                        stop=(kt == DT - 1),
                    )
                nc.scalar.activation(
                    hsb[0:RANK, :],
                    psh[0:RANK, 0:TO],
                    Act.Silu,
                    bias=sgc[0:RANK, k : k + 1],
                )
                for md in range(DT):
                    py = py_tile()
                    nc.tensor.matmul(
                        py[:],
                        lhsT=uw_sb[0:RANK, md * 128 : (md + 1) * 128],
                        rhs=hsb[0:RANK, :],
                        start=True,
                        stop=True,
                    )
                    nc.scalar.activation(
                        tgf[:, md * TO : (md + 1) * TO],
                        py[:],
                        Act.Tanh,
                        scale=0.5,
                        bias=sgc[:, 4 + k * 4 + md : 5 + k * 4 + md],
                    )
                # blend: xA = old + (0.5 + 0.5*t) * (mixed - old)
                dfs = lay.tile([128, DT * TO], bf, tag="dfs", name="dfs", bufs=1)
                nc.vector.tensor_tensor(
                    out=dfs[:], in0=mixed[:], in1=old_tile[:], op=Alu.subtract
                )
                nc.vector.tensor_scalar(tgf[:], tgf[:], 0.5, 0.5, Alu.mult, Alu.add)
                nc.gpsimd.tensor_tensor(out=dfs[:], in0=tgf[:], in1=dfs[:], op=Alu.mult)
                nc.vector.tensor_tensor(
                    out=xA[:], in0=old_tile[:], in1=dfs[:], op=Alu.add
                )

            # ---------------- run the model ----------------
            gated2 = None
            for si in range(N_STACKS):
                run_stack([0, 1, 2][si])
                if si == 0:
                    boundary(0, initial)
                elif si == 1:
                    boundary(1, initial)
                    gated2 = keep_tile()  # initial dead
                    nc.vector.tensor_scalar(gated2[:], xA[:], 1.0, None, Alu.mult)
                elif si == 2:
                    boundary(2, gated2)

            if DEBUG_TRUNK_OUT:
                nc.sync.dma_start(d_trunk[:], xA[:])

            # ---------------- final rmsnorm + AG + tied head ----------------
            if not SKIP_HEAD:
                hd = ctx.enter_context(tc.tile_pool(name="hd", bufs=1))
                ob_own = hd.tile([128, DT * TO], bf, tag="obo", name="obo")
                sq = t_sq()
                nc.scalar.activation(sq[:], xA[:], Act.Square)
                pn = py_tile()
                for kt in range(DT):
                    nc.tensor.matmul(
                        pn[0:1, :],
                        lhsT=ones_bf[:],
                        rhs=sq[:, kt * TO : (kt + 1) * TO],
                        start=(kt == 0),
                        stop=(kt == DT - 1),
                    )
                srow = t_srow()
                nc.scalar.activation(
                    srow[:], pn[0:1, :], Act.Abs_reciprocal_sqrt,
                    bias=epsc[0:1, :], scale=1.0 / D,
                )
                sbc = t_sbc()
                nc.gpsimd.partition_broadcast(sbc[:], srow[0:1, :])
                for dtt in range(DT):
                    nc.vector.tensor_tensor(
                        out=ob_own[:, dtt * TO : (dtt + 1) * TO],
                        in0=xA[:, dtt * TO : (dtt + 1) * TO],
                        in1=sbc[:],
                        op=Alu.mult,
                    )
                agi, ago = agiT[256], agoT[256]
                nc.sync.dma_start(agi[:], ob_own[:])
                nc.gpsimd.collective_compute(
                    "AllGather",
                    mybir.AluOpType.bypass,
                    replica_groups=RG,
                    ins=[agi[:]],
                    outs=[ago[0 : 4 * 128, :]],
                )
                ob = hd.tile([128, DT * T], bf, tag="obf", name="obf")
                ob3 = ob[:].rearrange("p (k t) -> p k t", k=DT)
                for r in range(4):
                    nc.sync.dma_start(
                        ob3[:, :, r * TO : (r + 1) * TO],
                        ago[r * 128 : (r + 1) * 128, :].rearrange(
                            "p (k t) -> p k t", k=DT
                        ),
                    )
                for nv in range(NV):
                    nw = min(512, VS - nv * 512)
                    rhsb = hd.tile([128, DT * 512], bf, tag="rhsb", name="rhsb", bufs=3)
                    nc.sync.dma_start(rhsb[:], d_embT[nv])
                    pcb = pc_tile()
                    for mt in range(8):
                        r = mt % 4
                        if r == 0:
                            psl = pg_tile()
                        elif r == 1:
                            psl = pu_tile()
                        else:
                            psl = pcb[:, 0:512] if r == 2 else pcb[:, 512:1024]
                        for kt in range(DT):
                            nc.tensor.matmul(
                                psl[:, :nw],
                                lhsT=ob[:, kt * T + mt * 128 : kt * T + (mt + 1) * 128],
                                rhs=rhsb[:, kt * 512 : kt * 512 + nw],
                                start=(kt == 0),
                                stop=(kt == DT - 1),
                            )
                        lsb = hd.tile([128, 512], f32, tag="lsb", name="lsb", bufs=4)
                        if mt % 2 == 0:
                            nc.scalar.activation(lsb[:, :nw], psl[:, :nw], Act.Copy)
                        else:
                            nc.vector.tensor_copy(lsb[:, :nw], psl[:, :nw])
                        outq = nc.gpsimd if mt % 2 == 0 else nc.sync
                        outq.dma_start(
                            d_out[mt * 128 : (mt + 1) * 128, nv * 512 : nv * 512 + nw],
                            lsb[:, :nw],
                        )

    nc.finalize()
    _prog_cache[key] = nc
    return nc


def prep_inputs(inputs):
    """Host-side: full model inputs -> list of 8 per-core in_maps."""
    idx = np.asarray(inputs["idx"])
    emb = _f32(inputs["emb"])
    pos = _f32(inputs["pos"])[0, :T]  # [T, D]
    we = _f32(inputs["emb_norm_w"])
    ts = _f32(inputs["token_shift"])
    mgw = _f32(inputs["mem_gate_w"])
    mgb = _f32(inputs["mem_gate_b"])
    memp = _f32(inputs["memory_p"])
    fnw = _f32(inputs["final_norm_w"])

    mgwT = np.ascontiguousarray(mgw.T)

    stack_in = {}
    for s in ("up", "dn"):
        nw = _f32(inputs[f"{s}_norm_w"])  # [NL, D]
        cw = _f32(inputs[f"{s}_conv_w"])[:, :, 0, :]  # [NL, D, K]
        cb = _f32(inputs[f"{s}_conv_b"])  # [NL, D]
        w1 = _f32(inputs[f"{s}_w1"])
        w2 = _f32(inputs[f"{s}_w2"])
        w3 = _f32(inputs[f"{s}_w3"])
        kp = _f32(inputs[f"{s}_kp"])
        ki = _f32(inputs[f"{s}_ki"])
        gn = _f32(inputs[f"{s}_gnorm"])
        cwb = np.zeros((NL, 128, DT), np.float32)
        cwt = np.zeros((NL, 128, KK), np.float32)
        cdiag = np.zeros((NL, 128, DT * KK * 128), np.float32)
        gdiag = np.zeros((NL, 128, DT * 128), np.float32)
        for li in range(NL):
            # conv path: gnorm cancels inside the double rmsnorm; fold norm_w
            # only. gnorm survives only in the residual base (gdiag).
            gfold = gn[li - 1] if li > 0 else np.ones(D, np.float32)
            cwf = cw[li] * nw[li][:, None]  # [D, K]
            taps = cwf[:, ::-1]  # tap m multiplies shift m*d
            cwb[li] = _cols(cb[li])
            cwt[li] = taps[3 * 128 : 4 * 128, :]
            for dtt in range(DT):
                for m in range(KK):
                    cdiag[li, :, (dtt * KK + m) * 128 : (dtt * KK + m + 1) * 128] = np.diag(
                        taps[dtt * 128 : (dtt + 1) * 128, m]
                    )
                gdiag[li, :, dtt * 128 : (dtt + 1) * 128] = np.diag(
                    gfold[dtt * 128 : (dtt + 1) * 128]
                )
        pid = np.zeros((128, (NL - 1) * 12), np.float32)
        for li in range(1, NL):
            pb = (li - 1) * 12
            pid[:, pb : pb + 4] = _cols(kp[li - 1] + ki[li - 1] / li)
            pid[:, pb + 4 : pb + 8] = _cols(ki[li - 1] / li)
        stack_in[f"{s}_cwb"] = np.ascontiguousarray(cwb)
        stack_in[f"{s}_cwt"] = np.ascontiguousarray(cwt)
        stack_in[f"{s}_pid"] = pid
        stack_in[f"{s}_cdiag"] = np.ascontiguousarray(cdiag).astype(BF16)
        stack_in[f"{s}_gdiag"] = np.ascontiguousarray(gdiag).astype(BF16)
        w13 = np.concatenate([w1.transpose(0, 2, 1), w3.transpose(0, 2, 1)], axis=2)
        w13p = np.ascontiguousarray(
            w13.reshape(NL, DT, 128, 2 * HID).transpose(0, 2, 1, 3).reshape(NL, 128, DT * 2 * HID)
        )
        stack_in[f"{s}_w13p"] = w13p.astype(BF16)
        w2T = w2.transpose(0, 2, 1)  # [NL, HID, D]
        w2p = np.ascontiguousarray(
            w2T.reshape(NL, HT, 128, D).transpose(0, 2, 1, 3).reshape(NL, 128, HT * D)
        )
        stack_in[f"{s}_w2p"] = w2p.astype(BF16)

    sgn = _f32(inputs["sg_norm"])
    sgdw = _f32(inputs["sg_down_w"])
    sgdb = _f32(inputs["sg_down_b"])
    sguw = _f32(inputs["sg_up_w"])
    sgub = _f32(inputs["sg_up_b"])
    dwT = np.stack(
        [np.ascontiguousarray(sgdw[k].T * sgn[k][:, None]) for k in range(3)]
    ).astype(BF16)
    uwT = np.stack([np.ascontiguousarray(sguw[k].T) for k in range(3)]).astype(BF16)
    sgc = np.zeros((128, 16), np.float32)
    for k in range(3):
        sgc[0:RANK, k] = sgdb[k]
        sgc[:, 4 + k * 4 : 8 + k * 4] = _cols(0.5 * sgub[k])

    embT = np.ascontiguousarray((emb.T * fnw[:, None]))  # [D, V] f32

    common = dict(
        mgwT=mgwT,
        emb_tbl=emb,
        sg_dwT=dwT,
        sg_uwT=uwT,
        sg_cols=sgc,
        **stack_in,
    )

    in_maps = []
    for c in range(NCORES):
        b = c // 4
        q = c % 4
        beta = q * TO
        m = dict(common)
        # idx: 3 chunks of 128 tokens: [beta-128, beta+256); q=0 pads with idx[0]
        tok = np.arange(beta - 128, beta + TO)
        tokc = np.clip(tok, 0, T - 1)
        m["idx_rs"] = np.ascontiguousarray(
            idx[b][tokc].astype(np.int32).reshape(3, 128).T
        )
        m["pos_rs"] = np.ascontiguousarray(
            pos[tokc].reshape(3, 128, D).transpose(1, 0, 2).reshape(128, 3 * D)
        )
        cst = np.zeros((128, 64), np.float32)
        # col0 coefs: q=0 -> shifted[0]=x[0]: cself0=we, cprev0=0
        if q == 0:
            cst[:, 0:4] = _cols(we)
            cst[:, 12:16] = 0.0
        else:
            cst[:, 0:4] = _cols((1.0 - ts) * we)
            cst[:, 12:16] = _cols(ts * we)
        cst[:, 4:8] = _cols(ts * we)
        cst[:, 8:12] = _cols((1.0 - ts) * we)
        cst[:, 16:20] = _cols(0.5 * mgb)
        cst[:, 20:24] = _cols(memp[b])
        for r in range(4):
            cst[:, 24 + r] = 1.0 if r < q else 0.0
        m["cst"] = cst
        rc = (MIX_W / (np.arange(beta + 1, beta + TO + 1, dtype=np.float32)))[None, :]
        m["rc_bc"] = np.ascontiguousarray(np.broadcast_to(rc, (128, TO)))
        # halo row-offset tables: rows into ago [640, ...]; block 4 is zeros
        offs = np.zeros((128, 2), np.int32)
        p = np.arange(128, dtype=np.int32)
        offs[:, 0] = (q - 1) * 128 + p if q >= 1 else 4 * 128 + p
        offs[:, 1] = (q - 2) * 128 + p if q >= 2 else 4 * 128 + p
        m["halo_offs"] = offs
        # head vocab shard
        esh = embT[:, q * VS : (q + 1) * VS]  # [D, VS]
        eshpad = np.zeros((D, NV * 512), np.float32)
        eshpad[:, :VS] = esh
        eshp = np.ascontiguousarray(
            eshpad.reshape(DT, 128, NV, 512).transpose(2, 1, 0, 3).reshape(NV, 128, DT * 512)
        )
        m["embT_sh"] = eshp.astype(BF16)
        in_maps.append(m)
    return in_maps


LAST_RESULTS = None


def kernel(**inputs):
    global LAST_RESULTS
    from concourse.bass_utils import run_bass_kernel_spmd

    nc = build_program()
    in_maps = prep_inputs(inputs)
    trace = bool(int(os.environ.get("KB_TRACE", "0")))
    res = run_bass_kernel_spmd(nc, in_maps, core_ids=list(range(NCORES)), trace=trace)
    LAST_RESULTS = res
    out = np.zeros((B, T, VOCAB), np.float32)
    for c in range(NCORES):
        b = c // 4
        q = c % 4
        out[b, :, q * VS : (q + 1) * VS] = res.results[c]["logits_sh"]
    return out
